# revision 2
# baseline (speedup 1.0000x reference)
"""GAT layer kernel for Trainium2 (Bass/Tile), data-parallel over batch on 8 cores.

v9: histogram-gather reformulation — never materializes the [N,N] mask.

Per-core math (batch item b, N=2048, F=128, K=128 grid):
    s = x @ (W @ w_mlp) + b;  p = exp(s);  q = exp(0.2 s)
    grid t_k = -m + k*(2m/K), m = 1.01*max|s|
    O1[j,k] = [s_j > t_k]      (N x K comparisons)
    O2[k,i] = [s_i <= -t_k]    (K x N comparisons)
    F_w[k]  = sum_j O1[j,k] w_j            (tiny matmul, w in {p,q,r,u})
    (Mw)_i  ~= sum_k O2[k,i] (F_w[k]-F_w[k-1])   (gather matmul)
    D = p*(Mp) + q*(Qtot-(Mq));  r = p/D; u = q/D
    col = p*(Mr) + q*(Utot-(Mu));  out = lrelu(h) * col,  h = x @ W
Threshold quantization flips mask entries only in a |s_i+s_j| < delta band
where exp(lrelu(.)) is branch-insensitive, so the error is O(delta^2).

Phase layout minimizes cross-engine round trips: all PE transposes first
(casts chase on DVE/ACT), then s-matmuls (s ready early), h-matmuls fill
the PE while DVE builds the grid + O1/O2 comparisons.
"""

import sys

if "/opt/trn_rl_repo" not in sys.path:
    sys.path.insert(0, "/opt/trn_rl_repo")

from contextlib import ExitStack

import numpy as np

import concourse.bass as bass
import concourse.bass_isa as bass_isa
import concourse.mybir as mybir
import concourse.tile as tile
from concourse import bacc
from concourse import masks
from concourse.bass_utils import run_bass_kernel_spmd

B, N, F = 8, 2048, 128
NB = N // 128  # 16 token blocks
K = 128        # threshold grid size
NEG_SLOPE = 0.2
FP32 = mybir.dt.float32
BF16 = mybir.dt.bfloat16
ALU = mybir.AluOpType
AFT = mybir.ActivationFunctionType


def gat_kernel(ctx, tc, out_d, x_d, W_d, wm_d, bm_d, iota_d, c42_d):
    nc = tc.nc

    const_p = ctx.enter_context(tc.tile_pool(name="const", bufs=1))
    big_p = ctx.enter_context(tc.tile_pool(name="big", bufs=1))
    vec_p = ctx.enter_context(tc.tile_pool(name="vec", bufs=1))
    outsb_p = ctx.enter_context(tc.tile_pool(name="outsb", bufs=4))
    # PSUM: 8 banks. big=4 (h_ps, held to the end), tr=3 rotating, sm=1.
    ps_big = ctx.enter_context(tc.tile_pool(name="ps_big", bufs=1, space="PSUM"))
    ps_tr = ctx.enter_context(tc.tile_pool(name="ps_tr", bufs=3, space="PSUM"))
    ps_sm = ctx.enter_context(tc.tile_pool(name="ps_sm", bufs=1, space="PSUM"))

    # ---------------- input DMAs + gpsimd consts first ------------------
    W_sb = const_p.tile([128, 128], FP32, tag="W_sb")
    nc.sync.dma_start(W_sb[:], W_d[:, :])
    ident_f = const_p.tile([128, 128], FP32, tag="ident_f")
    ident_b = const_p.tile([128, 128], BF16, tag="ident_b")
    masks.make_identity(nc, ident_f[:])
    masks.make_identity(nc, ident_b[:])
    ones_f = const_p.tile([128, 1], FP32, tag="ones_f")
    nc.gpsimd.memset(ones_f[:], 1.0)
    ones_row_f = const_p.tile([1, 128], FP32, tag="ones_row_f")
    nc.gpsimd.memset(ones_row_f[:], 1.0)
    ones_row_b = const_p.tile([1, 128], BF16, tag="ones_row_b")
    nc.gpsimd.memset(ones_row_b[:], 1.0)
    neg_ones_row_f = const_p.tile([1, 128], FP32, tag="neg_ones_row_f")
    nc.gpsimd.memset(neg_ones_row_f[:], -1.0)

    x_view = x_d.rearrange("(t p) f -> p t f", p=128)
    x_sb = big_p.tile([128, NB, 128], FP32, tag="x_sb")
    x_q_of = {2: nc.scalar, 8: nc.scalar, 14: nc.scalar}
    x_alt = [nc.sync, nc.gpsimd]
    n_alt = 0
    for t in range(NB):
        q = x_q_of.get(t)
        if q is None:
            q = x_alt[n_alt % 2]
            n_alt += 1
        q.dma_start(x_sb[:, t, :], x_view[:, t, :])
    wm_sb = const_p.tile([128, 1], FP32, tag="wm_sb")
    nc.scalar.dma_start(wm_sb[:], wm_d.rearrange("(p o) -> p o", o=1))
    b_sb = const_p.tile([1, 1], FP32, tag="b_sb")
    nc.scalar.dma_start(b_sb[:], bm_d.rearrange("(p o) -> p o", o=1))
    iota_col = const_p.tile([128, 1], FP32, tag="iota_col")
    nc.scalar.dma_start(iota_col[:], iota_d.rearrange("(p o) -> p o", o=1))
    iota_row = const_p.tile([1, 128], FP32, tag="iota_row")
    nc.scalar.dma_start(iota_row[:], iota_d.rearrange("(o k) -> o k", o=1))
    C42 = const_p.tile([4, 2], FP32, tag="C42")
    nc.scalar.dma_start(C42[:], c42_d[:, :])

    # ACT tables (Exp for p/q, Relu for the output stage)
    warm = const_p.tile([128, 1], FP32, tag="warm")
    nc.scalar.activation(warm[:], ones_f[:], AFT.Exp)
    nc.scalar.activation(warm[:], ones_f[:], AFT.Relu)

    # b broadcast to [128,1] via K=1 PE matmul
    b_ps = ps_sm.tile([128, 1], FP32, tag="sm")
    nc.tensor.matmul(b_ps[:], lhsT=ones_row_f[:], rhs=b_sb[:], start=True, stop=True)
    b_bc = const_p.tile([128, 1], FP32, tag="b_bc")
    nc.vector.tensor_copy(b_bc[:], b_ps[:])

    # ---------------- W -> bf16; v = W @ w_mlp; vk pair ------------------
    W_hi = const_p.tile([128, 128], BF16, tag="W_hi")
    nc.scalar.copy(W_hi[:], W_sb[:])
    WT_ps = ps_sm.tile([128, 128], FP32, tag="sm")
    nc.tensor.transpose(WT_ps[:], W_sb[:], ident_f[:])
    WT_sb = vec_p.tile([128, 128], FP32, tag="WT_sb")
    nc.vector.tensor_copy(WT_sb[:], WT_ps[:])
    v_ps = ps_sm.tile([128, 1], FP32, tag="sm")
    nc.tensor.matmul(v_ps[:], lhsT=WT_sb[:], rhs=wm_sb[:], start=True, stop=True)
    v_sb = vec_p.tile([128, 1], FP32, tag="v_sb")
    nc.vector.tensor_copy(v_sb[:], v_ps[:])
    vk = vec_p.tile([128, 2], BF16, tag="vk")
    nc.vector.tensor_copy(vk[:, 0:1], v_sb[:])
    v_hi32 = vec_p.tile([128, 1], FP32, tag="v_hi32")
    nc.vector.tensor_copy(v_hi32[:], vk[:, 0:1])
    nc.vector.tensor_tensor(vk[:, 1:2], v_sb[:], v_hi32[:], ALU.subtract)

    # ---------------- phase A: all transposes, casts chase ---------------
    xT_hi = big_p.tile([128, N], BF16, tag="xT_hi")  # [f, tok]
    for t in range(NB):
        sl = slice(t * 128, (t + 1) * 128)
        tr32 = ps_tr.tile([128, 128], FP32, tag="trb")
        nc.tensor.transpose(tr32[:], x_sb[:, t, :], ident_f[:])
        nc.vector.tensor_copy(xT_hi[:, sl], tr32[:])

    # ---------------- phase B: s matmuls, s assembly ---------------------
    s4_ps = ps_sm.tile([128, NB, 2], FP32, tag="sm")
    for t in range(NB):
        sl = slice(t * 128, (t + 1) * 128)
        nc.tensor.matmul(
            s4_ps[:, t, :], lhsT=xT_hi[:, sl], rhs=vk[:], start=True, stop=True
        )
    s4_sb = vec_p.tile([128, NB, 2], FP32, tag="s4_sb")
    nc.vector.tensor_copy(s4_sb[:], s4_ps[:])
    s12 = vec_p.tile([128, NB], FP32, tag="s12")
    nc.vector.tensor_tensor(s12[:], s4_sb[:, :, 0], s4_sb[:, :, 1], ALU.add)
    s_mat = vec_p.tile([128, NB], FP32, tag="s_mat")
    nc.vector.tensor_scalar(s_mat[:], s12[:], b_bc[:, 0:1], None, ALU.add)
    s_hi = vec_p.tile([128, NB], BF16, tag="s_hi")
    nc.vector.tensor_copy(s_hi[:], s_mat[:])
    sT_ps = ps_tr.tile([16, 128], BF16, tag="trb")
    nc.tensor.transpose(sT_ps[:], s_hi[:], ident_b[:])
    sT_sb = vec_p.tile([16, 128], BF16, tag="sT_sb")
    nc.vector.tensor_copy(sT_sb[:], sT_ps[:])
    s_flat = vec_p.tile([1, N], BF16, tag="s_flat")
    nc.sync.dma_start(s_flat[0:1, :], sT_sb[:, :])

    # grid: m = 1.01*max|s|, thresholds along free (Tgrid) and partitions
    amax = vec_p.tile([128, 1], FP32, tag="amax")
    nc.vector.tensor_reduce(
        amax[:], s_mat[:], axis=mybir.AxisListType.X, op=ALU.max,
        apply_absolute_value=True,
    )
    m_all = vec_p.tile([128, 1], FP32, tag="m_all")
    nc.gpsimd.partition_all_reduce(
        m_all[:], amax[:], channels=128, reduce_op=bass_isa.ReduceOp.max
    )
    m_col = vec_p.tile([128, 1], FP32, tag="m_col")
    nc.vector.tensor_scalar(m_col[:], m_all[:], 1.01, None, ALU.mult)
    negm_col = vec_p.tile([128, 1], FP32, tag="negm_col")
    nc.vector.tensor_scalar(negm_col[:], m_col[:], -1.0, None, ALU.mult)
    delta_col = vec_p.tile([128, 1], FP32, tag="delta_col")
    nc.vector.tensor_scalar(delta_col[:], m_col[:], 2.0 / K, None, ALU.mult)
    negdelta_col = vec_p.tile([128, 1], FP32, tag="negdelta_col")
    nc.vector.tensor_scalar(negdelta_col[:], m_col[:], -2.0 / K, None, ALU.mult)
    negt_col = vec_p.tile([128, 1], FP32, tag="negt_col")
    nc.vector.tensor_scalar(
        negt_col[:], iota_col[:], negdelta_col[:, 0:1], m_col[:, 0:1],
        ALU.mult, ALU.add,
    )
    iota_bc_ps = ps_tr.tile([128, 128], FP32, tag="trb")
    nc.tensor.matmul(
        iota_bc_ps[:], lhsT=ones_row_f[:], rhs=iota_row[:], start=True, stop=True
    )
    Tgrid_row = vec_p.tile([128, K], FP32, tag="Tgrid_row")
    nc.vector.tensor_scalar(
        Tgrid_row[:], iota_bc_ps[:], delta_col[:, 0:1], negm_col[:, 0:1],
        ALU.mult, ALU.add,
    )

    # exps + Pk stationaries [p_hi,p_lo,q_hi,q_lo]
    p_v = vec_p.tile([128, NB], FP32, tag="p_v")
    nc.scalar.activation(p_v[:], s_mat[:], AFT.Exp)
    q_v = vec_p.tile([128, NB], FP32, tag="q_v")
    nc.scalar.activation(q_v[:], s_mat[:], AFT.Exp, scale=NEG_SLOPE)
    Pk = vec_p.tile([128, NB, 4], BF16, tag="Pk")
    nc.vector.tensor_copy(Pk[:, :, 0], p_v[:])
    p_hi32 = vec_p.tile([128, NB], FP32, tag="p_hi32")
    nc.vector.tensor_copy(p_hi32[:], Pk[:, :, 0])
    nc.vector.tensor_tensor(Pk[:, :, 1], p_v[:], p_hi32[:], ALU.subtract)
    nc.vector.tensor_copy(Pk[:, :, 2], q_v[:])
    q_hi32 = vec_p.tile([128, NB], FP32, tag="q_hi32")
    nc.vector.tensor_copy(q_hi32[:], Pk[:, :, 2])
    nc.vector.tensor_tensor(Pk[:, :, 3], q_v[:], q_hi32[:], ALU.subtract)

    # ---------------- phase C: S broadcasts (to SBUF), O1 + F1 -----------
    O1 = big_p.tile([128, NB, K], BF16, tag="O1")
    O2 = big_p.tile([128, NB, 128], BF16, tag="O2")
    S_row = big_p.tile([128, NB, 128], BF16, tag="S_row")
    for c in range(4):
        S_ps = ps_tr.tile([128, 512], FP32, tag="trb")
        nc.tensor.matmul(
            S_ps[:], lhsT=ones_row_b[:], rhs=s_flat[0:1, c * 512 : (c + 1) * 512],
            start=True, stop=True,
        )
        nc.scalar.copy(S_row[:, 4 * c : 4 * (c + 1), :], S_ps[:])
    F1_ps = ps_sm.tile([4, K], FP32, tag="sm")
    for a in range(NB):
        nc.vector.tensor_scalar(
            O1[:, a, :], Tgrid_row[:], s_mat[:, a : a + 1], None, ALU.is_lt
        )
        nc.tensor.matmul(
            F1_ps[:], lhsT=Pk[:, a, :], rhs=O1[:, a, :],
            start=(a == 0), stop=(a == NB - 1),
        )

    # h matmuls fill the PE behind the F1 sprint; h copied out of PSUM so
    # the output stage reads SBUF from both engines without serializing
    h_ps = ps_big.tile([128, NB, 128], FP32, tag="bigps")
    for t in range(NB):
        sl = slice(t * 128, (t + 1) * 128)
        nc.tensor.matmul(
            h_ps[:, t, :], lhsT=xT_hi[:, sl], rhs=W_hi[:], start=True, stop=True
        )
    h_sb = big_p.tile([128, NB, 128], FP32, tag="h_sb")
    for i2 in range(NB // 2):
        nc.scalar.copy(h_sb[:, 2 * i2 : 2 * i2 + 2, :], h_ps[:, 2 * i2 : 2 * i2 + 2, :])

    # Qtot / Ptot sums and broadcasts (needed at D assembly)
    qs = vec_p.tile([128, 1], FP32, tag="qs")
    nc.vector.reduce_sum(qs[:], q_v[:], axis=mybir.AxisListType.X)
    Qt_ps = ps_sm.tile([1, 1], FP32, tag="sm")
    nc.tensor.matmul(Qt_ps[:], lhsT=qs[:], rhs=ones_f[:, 0:1], start=True, stop=True)
    Qt_sb = vec_p.tile([1, 1], FP32, tag="Qt_sb")
    nc.vector.tensor_copy(Qt_sb[:], Qt_ps[:])
    nQt_ps = ps_sm.tile([128, 1], FP32, tag="sm")
    nc.tensor.matmul(nQt_ps[:], lhsT=neg_ones_row_f[:], rhs=Qt_sb[:], start=True, stop=True)
    negQt_col = vec_p.tile([128, 1], FP32, tag="negQt_col")
    nc.vector.tensor_copy(negQt_col[:], nQt_ps[:])

    # ---------------- D1 prep: diffs + transpose-combine -----------------
    def d_prep(F_ps, nm):
        F_sb = vec_p.tile([4, K], FP32, tag=nm + "F_sb")
        nc.vector.tensor_copy(F_sb[:], F_ps[:])
        D1_row = vec_p.tile([4, K], FP32, tag=nm + "D1_row")
        nc.vector.tensor_copy(D1_row[:, 0:1], F_sb[:, 0:1])
        nc.vector.tensor_tensor(
            D1_row[:, 1:K], F_sb[:, 1:K], F_sb[:, 0 : K - 1], ALU.subtract
        )
        Dcol_ps = ps_sm.tile([128, 2], FP32, tag="sm")
        nc.tensor.matmul(Dcol_ps[:], lhsT=D1_row[:], rhs=C42[:], start=True, stop=True)
        Dcol = vec_p.tile([128, 2], FP32, tag=nm + "Dcol")
        nc.vector.tensor_copy(Dcol[:], Dcol_ps[:])
        Dk = vec_p.tile([128, 4], BF16, tag=nm + "Dk")
        nc.vector.tensor_copy(Dk[:, 0:1], Dcol[:, 0:1])
        h32 = vec_p.tile([128, 2], FP32, tag=nm + "h32")
        nc.vector.tensor_copy(h32[:, 0:1], Dk[:, 0:1])
        nc.vector.tensor_tensor(Dk[:, 1:2], Dcol[:, 0:1], h32[:, 0:1], ALU.subtract)
        nc.vector.tensor_copy(Dk[:, 2:3], Dcol[:, 1:2])
        nc.vector.tensor_copy(h32[:, 1:2], Dk[:, 2:3])
        nc.vector.tensor_tensor(Dk[:, 3:4], Dcol[:, 1:2], h32[:, 1:2], ALU.subtract)
        return Dk

    Dk1 = d_prep(F1_ps, "g1")

    # ---------------- O2 comparisons -------------------------------------
    for c in range(4):
        nc.vector.tensor_scalar(
            O2[:, c * 4 : (c + 1) * 4, :],
            S_row[:, 4 * c : 4 * (c + 1), :],
            negt_col[:, 0:1], None, ALU.is_le,
        )

    # ---------------- gather 1: col-layout matmuls -----------------------
    MPq_ps = ps_sm.tile([128, NB, 4], FP32, tag="sm")
    for t in range(NB):
        nc.tensor.matmul(
            MPq_ps[:, t, :], lhsT=O2[:, t, :], rhs=Dk1[:], start=True, stop=True
        )
    Dp = vec_p.tile([128, NB, 4], FP32, tag="Dp")
    nc.vector.tensor_copy(Dp[:], MPq_ps[:])

    # ---------------- D assembly, r/u, Rk --------------------------------
    MP = vec_p.tile([128, NB], FP32, tag="MP")
    nc.vector.tensor_tensor(MP[:], Dp[:, :, 0], Dp[:, :, 1], ALU.add)
    MQ = vec_p.tile([128, NB], FP32, tag="MQ")
    nc.vector.tensor_tensor(MQ[:], Dp[:, :, 2], Dp[:, :, 3], ALU.add)
    t2 = vec_p.tile([128, NB], FP32, tag="t2")
    nc.vector.scalar_tensor_tensor(
        t2[:], MQ[:], negQt_col[:, 0:1], q_v[:], ALU.add, ALU.mult
    )
    t3 = vec_p.tile([128, NB], FP32, tag="t3")
    nc.vector.tensor_tensor(t3[:], p_v[:], MP[:], ALU.mult)
    D_v = vec_p.tile([128, NB], FP32, tag="D_v")
    nc.vector.tensor_tensor(D_v[:], t3[:], t2[:], ALU.subtract)
    invD = vec_p.tile([128, NB], FP32, tag="invD")
    nc.vector.reciprocal(invD[:], D_v[:])
    r_v = vec_p.tile([128, NB], FP32, tag="r_v")
    nc.vector.tensor_tensor(r_v[:], p_v[:], invD[:], ALU.mult)
    u_v = vec_p.tile([128, NB], FP32, tag="u_v")
    nc.vector.tensor_tensor(u_v[:], q_v[:], invD[:], ALU.mult)
    Rk = vec_p.tile([128, NB, 4], BF16, tag="Rk")
    nc.vector.tensor_copy(Rk[:, :, 0], r_v[:])
    r_hi32 = vec_p.tile([128, NB], FP32, tag="r_hi32")
    nc.vector.tensor_copy(r_hi32[:], Rk[:, :, 0])
    nc.vector.tensor_tensor(Rk[:, :, 1], r_v[:], r_hi32[:], ALU.subtract)
    nc.vector.tensor_copy(Rk[:, :, 2], u_v[:])
    u_hi32 = vec_p.tile([128, NB], FP32, tag="u_hi32")
    nc.vector.tensor_copy(u_hi32[:], Rk[:, :, 2])
    nc.vector.tensor_tensor(Rk[:, :, 3], u_v[:], u_hi32[:], ALU.subtract)
    us = vec_p.tile([128, 1], FP32, tag="us")
    nc.vector.reduce_sum(us[:], u_v[:], axis=mybir.AxisListType.X)

    # Utot scalar -> -Utot broadcast
    Ut_ps = ps_sm.tile([1, 1], FP32, tag="sm")
    nc.tensor.matmul(Ut_ps[:], lhsT=us[:], rhs=ones_f[:, 0:1], start=True, stop=True)
    Ut_sb = vec_p.tile([1, 1], FP32, tag="Ut_sb")
    nc.vector.tensor_copy(Ut_sb[:], Ut_ps[:])
    nUt_ps = ps_sm.tile([128, 1], FP32, tag="sm")
    nc.tensor.matmul(nUt_ps[:], lhsT=neg_ones_row_f[:], rhs=Ut_sb[:], start=True, stop=True)
    negUt_col = vec_p.tile([128, 1], FP32, tag="negUt_col")
    nc.vector.tensor_copy(negUt_col[:], nUt_ps[:])

    # ---------------- F2 matvec ------------------------------------------
    F2_ps = ps_sm.tile([4, K], FP32, tag="sm")
    for a in range(NB):
        nc.tensor.matmul(
            F2_ps[:], lhsT=Rk[:, a, :], rhs=O1[:, a, :],
            start=(a == 0), stop=(a == NB - 1),
        )
    Dk2 = d_prep(F2_ps, "g2")

    # ---------------- gather 2 + output ----------------------------------
    out_view = out_d.rearrange("(t p) f -> p t f", p=128)
    Gp_ps = ps_sm.tile([128, NB, 4], FP32, tag="sm")
    Gp = vec_p.tile([128, NB, 4], FP32, tag="Gp")
    for t in range(NB):
        nc.tensor.matmul(
            Gp_ps[:, t, :], lhsT=O2[:, t, :], rhs=Dk2[:], start=True, stop=True
        )
    nc.scalar.copy(Gp[:], Gp_ps[:])
    MR = vec_p.tile([128, NB], FP32, tag="MR")
    MU = vec_p.tile([128, NB], FP32, tag="MU")
    z2 = vec_p.tile([128, NB], FP32, tag="z2")
    z3 = vec_p.tile([128, NB], FP32, tag="z3")
    col = vec_p.tile([128, NB], FP32, tag="col")
    col08 = vec_p.tile([128, NB], FP32, tag="col08")
    col02 = vec_p.tile([128, NB], FP32, tag="col02")
    nc.vector.tensor_tensor(MR[:], Gp[:, :, 0], Gp[:, :, 1], ALU.add)
    nc.vector.tensor_tensor(MU[:], Gp[:, :, 2], Gp[:, :, 3], ALU.add)
    nc.vector.scalar_tensor_tensor(
        z2[:], MU[:], negUt_col[:, 0:1], q_v[:], ALU.add, ALU.mult
    )
    nc.vector.tensor_tensor(z3[:], p_v[:], MR[:], ALU.mult)
    nc.vector.tensor_tensor(col[:], z3[:], z2[:], ALU.subtract)
    nc.vector.tensor_scalar(col08[:], col[:], 0.8, None, ALU.mult)
    nc.vector.tensor_scalar(col02[:], col[:], 0.2, None, ALU.mult)
    # out = col*lrelu(h) = relu(0.8*col*h) + 0.2*col*h   (col > 0)
    # pairs interleave ACT relu and DVE combine for cross-engine overlap
    r1_all = big_p.tile([128, NB, 128], FP32, tag="r1_all")
    for g in range(NB // 2):
        for t in (2 * g, 2 * g + 1):
            nc.scalar.activation(
                r1_all[:, t, :], h_sb[:, t, :], AFT.Relu,
                scale=col08[:, t : t + 1],
            )
        for t in (2 * g, 2 * g + 1):
            o_sb = outsb_p.tile([128, 128], FP32, tag="o_sb")
            nc.vector.scalar_tensor_tensor(
                o_sb[:], h_sb[:, t, :], col02[:, t : t + 1], r1_all[:, t, :],
                ALU.mult, ALU.add,
            )
            (nc.sync if t % 2 == 0 else nc.gpsimd).dma_start(
                out_view[:, t, :], o_sb[:]
            )


def build_nc(num_devices: int = 8) -> "bass.Bass":
    nc = bacc.Bacc(
        "TRN2", target_bir_lowering=False, debug=False, num_devices=num_devices
    )
    x_d = nc.dram_tensor("x", [N, F], FP32, kind="ExternalInput")
    W_d = nc.dram_tensor("W", [F, F], FP32, kind="ExternalInput")
    wm_d = nc.dram_tensor("w_mlp", [F], FP32, kind="ExternalInput")
    bm_d = nc.dram_tensor("b_mlp", [1], FP32, kind="ExternalInput")
    iota_d = nc.dram_tensor("iota", [128], FP32, kind="ExternalInput")
    c42_d = nc.dram_tensor("c42", [4, 2], FP32, kind="ExternalInput")
    out_d = nc.dram_tensor("out", [N, F], FP32, kind="ExternalOutput")
    with tile.TileContext(nc) as tc:
        with ExitStack() as ctx:
            gat_kernel(
                ctx, tc, out_d.ap(), x_d.ap(), W_d.ap(), wm_d.ap(), bm_d.ap(),
                iota_d.ap(), c42_d.ap(),
            )
    nc.compile()
    return nc


_NC_CACHE: dict = {}


def run(x, W, w_mlp, b_mlp, trace=False, **spmd_kwargs):
    x = np.asarray(x, dtype=np.float32)
    W = np.asarray(W, dtype=np.float32)
    w_mlp = np.asarray(w_mlp, dtype=np.float32)
    b_mlp = np.asarray(b_mlp, dtype=np.float32)

    if "nc" not in _NC_CACHE:
        _NC_CACHE["nc"] = build_nc(num_devices=B)
    nc = _NC_CACHE["nc"]

    iota = np.arange(128, dtype=np.float32)
    c42 = np.array([[1, 0], [1, 0], [0, 1], [0, 1]], dtype=np.float32)
    in_maps = [
        {
            "x": np.ascontiguousarray(x[b, 0]),
            "W": W,
            "w_mlp": w_mlp,
            "b_mlp": b_mlp,
            "iota": iota,
            "c42": c42,
        }
        for b in range(B)
    ]
    res = run_bass_kernel_spmd(
        nc, in_maps, core_ids=list(range(B)), trace=trace, **spmd_kwargs
    )
    out = np.stack([res.results[b]["out"] for b in range(B)])[:, None]
    return out.astype(np.float32), res


def kernel(x, W, w_mlp, b_mlp):
    out, _ = run(x, W, w_mlp, b_mlp)
    return out


# revision 3
# speedup vs baseline: 1.0128x; 1.0128x over previous
"""GAT layer kernel for Trainium2 (Bass/Tile), data-parallel over batch on 8 cores.

v9: histogram-gather reformulation — never materializes the [N,N] mask.

Per-core math (batch item b, N=2048, F=128, K=128 grid):
    s = x @ (W @ w_mlp) + b;  p = exp(s);  q = exp(0.2 s)
    grid t_k = -m + k*(2m/K), m = 1.01*max|s|
    O1[j,k] = [s_j > t_k]      (N x K comparisons)
    O2[k,i] = [s_i <= -t_k]    (K x N comparisons)
    F_w[k]  = sum_j O1[j,k] w_j            (tiny matmul, w in {p,q,r,u})
    (Mw)_i  ~= sum_k O2[k,i] (F_w[k]-F_w[k-1])   (gather matmul)
    D = p*(Mp) + q*(Qtot-(Mq));  r = p/D; u = q/D
    col = p*(Mr) + q*(Utot-(Mu));  out = lrelu(h) * col,  h = x @ W
Threshold quantization flips mask entries only in a |s_i+s_j| < delta band
where exp(lrelu(.)) is branch-insensitive, so the error is O(delta^2).

Phase layout minimizes cross-engine round trips: all PE transposes first
(casts chase on DVE/ACT), then s-matmuls (s ready early), h-matmuls fill
the PE while DVE builds the grid + O1/O2 comparisons.
"""

import sys

if "/opt/trn_rl_repo" not in sys.path:
    sys.path.insert(0, "/opt/trn_rl_repo")

from contextlib import ExitStack

import numpy as np

import concourse.bass as bass
import concourse.bass_isa as bass_isa
import concourse.mybir as mybir
import concourse.tile as tile
from concourse import bacc
from concourse import masks
from concourse.bass_utils import run_bass_kernel_spmd

B, N, F = 8, 2048, 128
NB = N // 128  # 16 token blocks
K = 128        # threshold grid size
NEG_SLOPE = 0.2
FP32 = mybir.dt.float32
BF16 = mybir.dt.bfloat16
ALU = mybir.AluOpType
AFT = mybir.ActivationFunctionType


def gat_kernel(ctx, tc, out_d, x_d, W_d, wm_d, bm_d, iota_d, c42_d):
    nc = tc.nc

    const_p = ctx.enter_context(tc.tile_pool(name="const", bufs=1))
    big_p = ctx.enter_context(tc.tile_pool(name="big", bufs=1))
    vec_p = ctx.enter_context(tc.tile_pool(name="vec", bufs=1))
    outsb_p = ctx.enter_context(tc.tile_pool(name="outsb", bufs=6))
    # PSUM: 8 banks. big=4 (h_ps, held to the end), tr=3 rotating, sm=1.
    ps_big = ctx.enter_context(tc.tile_pool(name="ps_big", bufs=1, space="PSUM"))
    ps_tr = ctx.enter_context(tc.tile_pool(name="ps_tr", bufs=3, space="PSUM"))
    ps_sm = ctx.enter_context(tc.tile_pool(name="ps_sm", bufs=1, space="PSUM"))

    # ---------------- input DMAs + gpsimd consts first ------------------
    W_sb = const_p.tile([128, 128], FP32, tag="W_sb")
    nc.sync.dma_start(W_sb[:], W_d[:, :])
    ident_f = const_p.tile([128, 128], FP32, tag="ident_f")
    ident_b = const_p.tile([128, 128], BF16, tag="ident_b")
    masks.make_identity(nc, ident_f[:])
    masks.make_identity(nc, ident_b[:])
    ones_f = const_p.tile([128, 1], FP32, tag="ones_f")
    nc.gpsimd.memset(ones_f[:], 1.0)
    ones_row_f = const_p.tile([1, 128], FP32, tag="ones_row_f")
    nc.gpsimd.memset(ones_row_f[:], 1.0)
    ones_row_b = const_p.tile([1, 128], BF16, tag="ones_row_b")
    nc.gpsimd.memset(ones_row_b[:], 1.0)
    neg_ones_row_f = const_p.tile([1, 128], FP32, tag="neg_ones_row_f")
    nc.gpsimd.memset(neg_ones_row_f[:], -1.0)

    x_view = x_d.rearrange("(t p) f -> p t f", p=128)
    x_sb = big_p.tile([128, NB, 128], FP32, tag="x_sb")
    x_q_of = {2: nc.scalar, 8: nc.scalar, 14: nc.scalar}
    x_alt = [nc.sync, nc.gpsimd]
    n_alt = 0
    for t in range(NB):
        q = x_q_of.get(t)
        if q is None:
            q = x_alt[n_alt % 2]
            n_alt += 1
        q.dma_start(x_sb[:, t, :], x_view[:, t, :])
    wm_sb = const_p.tile([128, 1], FP32, tag="wm_sb")
    nc.scalar.dma_start(wm_sb[:], wm_d.rearrange("(p o) -> p o", o=1))
    b_sb = const_p.tile([1, 1], FP32, tag="b_sb")
    nc.scalar.dma_start(b_sb[:], bm_d.rearrange("(p o) -> p o", o=1))
    iota_col = const_p.tile([128, 1], FP32, tag="iota_col")
    nc.scalar.dma_start(iota_col[:], iota_d.rearrange("(p o) -> p o", o=1))
    iota_row = const_p.tile([1, 128], FP32, tag="iota_row")
    nc.scalar.dma_start(iota_row[:], iota_d.rearrange("(o k) -> o k", o=1))
    C42 = const_p.tile([4, 2], FP32, tag="C42")
    nc.scalar.dma_start(C42[:], c42_d[:, :])

    # ACT tables (Exp for p/q, Relu for the output stage)
    warm = const_p.tile([128, 1], FP32, tag="warm")
    nc.scalar.activation(warm[:], ones_f[:], AFT.Exp)
    nc.scalar.activation(warm[:], ones_f[:], AFT.Relu)

    # b broadcast to [128,1] via K=1 PE matmul
    b_ps = ps_sm.tile([128, 1], FP32, tag="sm")
    nc.tensor.matmul(b_ps[:], lhsT=ones_row_f[:], rhs=b_sb[:], start=True, stop=True)
    b_bc = const_p.tile([128, 1], FP32, tag="b_bc")
    nc.vector.tensor_copy(b_bc[:], b_ps[:])

    # ---------------- W -> bf16; v = W @ w_mlp; vk pair ------------------
    W_hi = const_p.tile([128, 128], BF16, tag="W_hi")
    nc.scalar.copy(W_hi[:], W_sb[:])
    WT_ps = ps_sm.tile([128, 128], FP32, tag="sm")
    nc.tensor.transpose(WT_ps[:], W_sb[:], ident_f[:])
    WT_sb = vec_p.tile([128, 128], FP32, tag="WT_sb")
    nc.vector.tensor_copy(WT_sb[:], WT_ps[:])
    v_ps = ps_sm.tile([128, 1], FP32, tag="sm")
    nc.tensor.matmul(v_ps[:], lhsT=WT_sb[:], rhs=wm_sb[:], start=True, stop=True)
    v_sb = vec_p.tile([128, 1], FP32, tag="v_sb")
    nc.vector.tensor_copy(v_sb[:], v_ps[:])
    vk = vec_p.tile([128, 2], BF16, tag="vk")
    nc.vector.tensor_copy(vk[:, 0:1], v_sb[:])
    v_hi32 = vec_p.tile([128, 1], FP32, tag="v_hi32")
    nc.vector.tensor_copy(v_hi32[:], vk[:, 0:1])
    nc.vector.tensor_tensor(vk[:, 1:2], v_sb[:], v_hi32[:], ALU.subtract)

    # ---------------- phase A: all transposes, casts chase ---------------
    xT_hi = big_p.tile([128, N], BF16, tag="xT_hi")  # [f, tok]
    for t in range(NB):
        sl = slice(t * 128, (t + 1) * 128)
        tr32 = ps_tr.tile([128, 128], FP32, tag="trb")
        nc.tensor.transpose(tr32[:], x_sb[:, t, :], ident_f[:])
        if t % 2 == 0:
            nc.vector.tensor_copy(xT_hi[:, sl], tr32[:])
        else:
            nc.scalar.copy(xT_hi[:, sl], tr32[:])

    # ---------------- phase B: s matmuls, s assembly ---------------------
    s4_ps = ps_sm.tile([128, NB, 2], FP32, tag="sm")
    for t in range(NB):
        sl = slice(t * 128, (t + 1) * 128)
        nc.tensor.matmul(
            s4_ps[:, t, :], lhsT=xT_hi[:, sl], rhs=vk[:], start=True, stop=True
        )
    s4_sb = vec_p.tile([128, NB, 2], FP32, tag="s4_sb")
    nc.vector.tensor_copy(s4_sb[:], s4_ps[:])
    s12 = vec_p.tile([128, NB], FP32, tag="s12")
    nc.vector.tensor_tensor(s12[:], s4_sb[:, :, 0], s4_sb[:, :, 1], ALU.add)
    s_mat = vec_p.tile([128, NB], FP32, tag="s_mat")
    nc.vector.tensor_scalar(s_mat[:], s12[:], b_bc[:, 0:1], None, ALU.add)
    s_hi = vec_p.tile([128, NB], BF16, tag="s_hi")
    nc.vector.tensor_copy(s_hi[:], s_mat[:])
    sT_ps = ps_tr.tile([16, 128], BF16, tag="trb")
    nc.tensor.transpose(sT_ps[:], s_hi[:], ident_b[:])
    sT_sb = vec_p.tile([16, 128], BF16, tag="sT_sb")
    nc.vector.tensor_copy(sT_sb[:], sT_ps[:])
    s_flat = vec_p.tile([1, N], BF16, tag="s_flat")
    nc.sync.dma_start(s_flat[0:1, :], sT_sb[:, :])

    # grid: m = 1.01*max|s|, thresholds along free (Tgrid) and partitions
    amax = vec_p.tile([128, 1], FP32, tag="amax")
    nc.vector.tensor_reduce(
        amax[:], s_mat[:], axis=mybir.AxisListType.X, op=ALU.max,
        apply_absolute_value=True,
    )
    amaxT_ps = ps_tr.tile([1, 128], FP32, tag="trb")
    nc.tensor.transpose(amaxT_ps[:], amax[:], ident_f[:])
    amaxT = vec_p.tile([1, 128], FP32, tag="amaxT")
    nc.vector.tensor_copy(amaxT[:], amaxT_ps[:])
    m_sc = vec_p.tile([1, 1], FP32, tag="m_sc")
    nc.vector.tensor_reduce(m_sc[:], amaxT[:], axis=mybir.AxisListType.X, op=ALU.max)
    m_ps = ps_sm.tile([128, 1], FP32, tag="sm")
    nc.tensor.matmul(m_ps[:], lhsT=ones_row_f[:], rhs=m_sc[:], start=True, stop=True)
    m_bc = vec_p.tile([128, 1], FP32, tag="m_bc")
    nc.vector.tensor_copy(m_bc[:], m_ps[:])
    m_col = vec_p.tile([128, 1], FP32, tag="m_col")
    nc.vector.tensor_scalar(m_col[:], m_bc[:], 1.01, None, ALU.mult)
    negm_col = vec_p.tile([128, 1], FP32, tag="negm_col")
    nc.vector.tensor_scalar(negm_col[:], m_col[:], -1.0, None, ALU.mult)
    delta_col = vec_p.tile([128, 1], FP32, tag="delta_col")
    nc.vector.tensor_scalar(delta_col[:], m_col[:], 2.0 / K, None, ALU.mult)
    negdelta_col = vec_p.tile([128, 1], FP32, tag="negdelta_col")
    nc.vector.tensor_scalar(negdelta_col[:], m_col[:], -2.0 / K, None, ALU.mult)
    negt_col = vec_p.tile([128, 1], FP32, tag="negt_col")
    nc.vector.tensor_scalar(
        negt_col[:], iota_col[:], negdelta_col[:, 0:1], m_col[:, 0:1],
        ALU.mult, ALU.add,
    )
    iota_bc_ps = ps_tr.tile([128, 128], FP32, tag="trb")
    nc.tensor.matmul(
        iota_bc_ps[:], lhsT=ones_row_f[:], rhs=iota_row[:], start=True, stop=True
    )
    Tgrid_row = vec_p.tile([128, K], FP32, tag="Tgrid_row")
    nc.vector.tensor_scalar(
        Tgrid_row[:], iota_bc_ps[:], delta_col[:, 0:1], negm_col[:, 0:1],
        ALU.mult, ALU.add,
    )

    # exps + Pk stationaries [p_hi,p_lo,q_hi,q_lo]
    p_v = vec_p.tile([128, NB], FP32, tag="p_v")
    nc.scalar.activation(p_v[:], s_mat[:], AFT.Exp)
    q_v = vec_p.tile([128, NB], FP32, tag="q_v")
    nc.scalar.activation(q_v[:], s_mat[:], AFT.Exp, scale=NEG_SLOPE)
    Pk = vec_p.tile([128, NB, 4], BF16, tag="Pk")
    nc.vector.tensor_copy(Pk[:, :, 0], p_v[:])
    p_hi32 = vec_p.tile([128, NB], FP32, tag="p_hi32")
    nc.vector.tensor_copy(p_hi32[:], Pk[:, :, 0])
    nc.vector.tensor_tensor(Pk[:, :, 1], p_v[:], p_hi32[:], ALU.subtract)
    nc.vector.tensor_copy(Pk[:, :, 2], q_v[:])
    q_hi32 = vec_p.tile([128, NB], FP32, tag="q_hi32")
    nc.vector.tensor_copy(q_hi32[:], Pk[:, :, 2])
    nc.vector.tensor_tensor(Pk[:, :, 3], q_v[:], q_hi32[:], ALU.subtract)

    # ---------------- phase C: S broadcasts (to SBUF), O1 + F1 -----------
    O1 = big_p.tile([128, NB, K], BF16, tag="O1")
    O2 = big_p.tile([128, NB, 128], BF16, tag="O2")
    S_row = big_p.tile([128, NB, 128], BF16, tag="S_row")
    for c in range(4):
        S_ps = ps_tr.tile([128, 512], FP32, tag="trb")
        nc.tensor.matmul(
            S_ps[:], lhsT=ones_row_b[:], rhs=s_flat[0:1, c * 512 : (c + 1) * 512],
            start=True, stop=True,
        )
        nc.scalar.copy(S_row[:, 4 * c : 4 * (c + 1), :], S_ps[:])
    F1_ps = ps_sm.tile([4, K], FP32, tag="sm")
    for a in range(NB):
        nc.vector.tensor_scalar(
            O1[:, a, :], Tgrid_row[:], s_mat[:, a : a + 1], None, ALU.is_lt
        )
        nc.tensor.matmul(
            F1_ps[:], lhsT=Pk[:, a, :], rhs=O1[:, a, :],
            start=(a == 0), stop=(a == NB - 1),
        )

    # h matmuls fill the PE behind the F1 sprint; h copied out of PSUM so
    # the output stage reads SBUF from both engines without serializing
    h_ps = ps_big.tile([128, NB, 128], FP32, tag="bigps")
    for t in range(NB):
        sl = slice(t * 128, (t + 1) * 128)
        nc.tensor.matmul(
            h_ps[:, t, :], lhsT=xT_hi[:, sl], rhs=W_hi[:], start=True, stop=True
        )
    h_sb = big_p.tile([128, NB, 128], FP32, tag="h_sb")
    for i2 in range(NB // 2):
        nc.scalar.copy(h_sb[:, 2 * i2 : 2 * i2 + 2, :], h_ps[:, 2 * i2 : 2 * i2 + 2, :])

    # Qtot / Ptot sums and broadcasts (needed at D assembly)
    qs = vec_p.tile([128, 1], FP32, tag="qs")
    nc.vector.reduce_sum(qs[:], q_v[:], axis=mybir.AxisListType.X)
    Qt_ps = ps_sm.tile([1, 1], FP32, tag="sm")
    nc.tensor.matmul(Qt_ps[:], lhsT=qs[:], rhs=ones_f[:, 0:1], start=True, stop=True)
    Qt_sb = vec_p.tile([1, 1], FP32, tag="Qt_sb")
    nc.vector.tensor_copy(Qt_sb[:], Qt_ps[:])
    nQt_ps = ps_sm.tile([128, 1], FP32, tag="sm")
    nc.tensor.matmul(nQt_ps[:], lhsT=neg_ones_row_f[:], rhs=Qt_sb[:], start=True, stop=True)
    negQt_col = vec_p.tile([128, 1], FP32, tag="negQt_col")
    nc.vector.tensor_copy(negQt_col[:], nQt_ps[:])

    # ---------------- D1 prep: diffs + transpose-combine -----------------
    def d_prep(F_ps, nm):
        F_sb = vec_p.tile([4, K], FP32, tag=nm + "F_sb")
        nc.vector.tensor_copy(F_sb[:], F_ps[:])
        D1_row = vec_p.tile([4, K], FP32, tag=nm + "D1_row")
        nc.vector.tensor_copy(D1_row[:, 0:1], F_sb[:, 0:1])
        nc.vector.tensor_tensor(
            D1_row[:, 1:K], F_sb[:, 1:K], F_sb[:, 0 : K - 1], ALU.subtract
        )
        Dcol_ps = ps_sm.tile([128, 2], FP32, tag="sm")
        nc.tensor.matmul(Dcol_ps[:], lhsT=D1_row[:], rhs=C42[:], start=True, stop=True)
        Dcol = vec_p.tile([128, 2], FP32, tag=nm + "Dcol")
        nc.vector.tensor_copy(Dcol[:], Dcol_ps[:])
        Dk = vec_p.tile([128, 4], BF16, tag=nm + "Dk")
        nc.vector.tensor_copy(Dk[:, 0:1], Dcol[:, 0:1])
        h32 = vec_p.tile([128, 2], FP32, tag=nm + "h32")
        nc.vector.tensor_copy(h32[:, 0:1], Dk[:, 0:1])
        nc.vector.tensor_tensor(Dk[:, 1:2], Dcol[:, 0:1], h32[:, 0:1], ALU.subtract)
        nc.vector.tensor_copy(Dk[:, 2:3], Dcol[:, 1:2])
        nc.vector.tensor_copy(h32[:, 1:2], Dk[:, 2:3])
        nc.vector.tensor_tensor(Dk[:, 3:4], Dcol[:, 1:2], h32[:, 1:2], ALU.subtract)
        return Dk

    Dk1 = d_prep(F1_ps, "g1")

    # ---------------- O2 comparisons -------------------------------------
    for c in range(4):
        nc.vector.tensor_scalar(
            O2[:, c * 4 : (c + 1) * 4, :],
            S_row[:, 4 * c : 4 * (c + 1), :],
            negt_col[:, 0:1], None, ALU.is_le,
        )

    # ---------------- gather 1: col-layout matmuls -----------------------
    MPq_ps = ps_sm.tile([128, NB, 4], FP32, tag="sm")
    for t in range(NB):
        nc.tensor.matmul(
            MPq_ps[:, t, :], lhsT=O2[:, t, :], rhs=Dk1[:], start=True, stop=True
        )
    Dp = vec_p.tile([128, NB, 4], FP32, tag="Dp")
    nc.vector.tensor_copy(Dp[:], MPq_ps[:])

    # ---------------- D assembly, r/u, Rk --------------------------------
    MP = vec_p.tile([128, NB], FP32, tag="MP")
    nc.vector.tensor_tensor(MP[:], Dp[:, :, 0], Dp[:, :, 1], ALU.add)
    MQ = vec_p.tile([128, NB], FP32, tag="MQ")
    nc.vector.tensor_tensor(MQ[:], Dp[:, :, 2], Dp[:, :, 3], ALU.add)
    t2 = vec_p.tile([128, NB], FP32, tag="t2")
    nc.vector.scalar_tensor_tensor(
        t2[:], MQ[:], negQt_col[:, 0:1], q_v[:], ALU.add, ALU.mult
    )
    t3 = vec_p.tile([128, NB], FP32, tag="t3")
    nc.vector.tensor_tensor(t3[:], p_v[:], MP[:], ALU.mult)
    D_v = vec_p.tile([128, NB], FP32, tag="D_v")
    nc.vector.tensor_tensor(D_v[:], t3[:], t2[:], ALU.subtract)
    invD = vec_p.tile([128, NB], FP32, tag="invD")
    nc.vector.reciprocal(invD[:], D_v[:])
    r_v = vec_p.tile([128, NB], FP32, tag="r_v")
    nc.vector.tensor_tensor(r_v[:], p_v[:], invD[:], ALU.mult)
    u_v = vec_p.tile([128, NB], FP32, tag="u_v")
    nc.vector.tensor_tensor(u_v[:], q_v[:], invD[:], ALU.mult)
    Rk = vec_p.tile([128, NB, 4], BF16, tag="Rk")
    nc.vector.tensor_copy(Rk[:, :, 0], r_v[:])
    r_hi32 = vec_p.tile([128, NB], FP32, tag="r_hi32")
    nc.vector.tensor_copy(r_hi32[:], Rk[:, :, 0])
    nc.vector.tensor_tensor(Rk[:, :, 1], r_v[:], r_hi32[:], ALU.subtract)
    nc.vector.tensor_copy(Rk[:, :, 2], u_v[:])
    u_hi32 = vec_p.tile([128, NB], FP32, tag="u_hi32")
    nc.vector.tensor_copy(u_hi32[:], Rk[:, :, 2])
    nc.vector.tensor_tensor(Rk[:, :, 3], u_v[:], u_hi32[:], ALU.subtract)
    us = vec_p.tile([128, 1], FP32, tag="us")
    nc.vector.reduce_sum(us[:], u_v[:], axis=mybir.AxisListType.X)

    # Utot scalar -> -Utot broadcast
    Ut_ps = ps_sm.tile([1, 1], FP32, tag="sm")
    nc.tensor.matmul(Ut_ps[:], lhsT=us[:], rhs=ones_f[:, 0:1], start=True, stop=True)
    Ut_sb = vec_p.tile([1, 1], FP32, tag="Ut_sb")
    nc.vector.tensor_copy(Ut_sb[:], Ut_ps[:])
    nUt_ps = ps_sm.tile([128, 1], FP32, tag="sm")
    nc.tensor.matmul(nUt_ps[:], lhsT=neg_ones_row_f[:], rhs=Ut_sb[:], start=True, stop=True)
    negUt_col = vec_p.tile([128, 1], FP32, tag="negUt_col")
    nc.vector.tensor_copy(negUt_col[:], nUt_ps[:])

    # ---------------- F2 matvec ------------------------------------------
    F2_ps = ps_sm.tile([4, K], FP32, tag="sm")
    for a in range(NB):
        nc.tensor.matmul(
            F2_ps[:], lhsT=Rk[:, a, :], rhs=O1[:, a, :],
            start=(a == 0), stop=(a == NB - 1),
        )
    Dk2 = d_prep(F2_ps, "g2")

    # ---------------- gather 2 + output ----------------------------------
    out_view = out_d.rearrange("(t p) f -> p t f", p=128)
    Gp_ps = ps_sm.tile([128, NB, 4], FP32, tag="sm")
    Gp = vec_p.tile([128, NB, 4], FP32, tag="Gp")
    for t in range(NB):
        nc.tensor.matmul(
            Gp_ps[:, t, :], lhsT=O2[:, t, :], rhs=Dk2[:], start=True, stop=True
        )
    nc.scalar.copy(Gp[:], Gp_ps[:])
    MR = vec_p.tile([128, NB], FP32, tag="MR")
    MU = vec_p.tile([128, NB], FP32, tag="MU")
    z2 = vec_p.tile([128, NB], FP32, tag="z2")
    z3 = vec_p.tile([128, NB], FP32, tag="z3")
    col = vec_p.tile([128, NB], FP32, tag="col")
    col08 = vec_p.tile([128, NB], FP32, tag="col08")
    col02 = vec_p.tile([128, NB], FP32, tag="col02")
    nc.vector.tensor_tensor(MR[:], Gp[:, :, 0], Gp[:, :, 1], ALU.add)
    nc.vector.tensor_tensor(MU[:], Gp[:, :, 2], Gp[:, :, 3], ALU.add)
    nc.vector.scalar_tensor_tensor(
        z2[:], MU[:], negUt_col[:, 0:1], q_v[:], ALU.add, ALU.mult
    )
    nc.vector.tensor_tensor(z3[:], p_v[:], MR[:], ALU.mult)
    nc.vector.tensor_tensor(col[:], z3[:], z2[:], ALU.subtract)
    nc.vector.tensor_scalar(col08[:], col[:], 0.8, None, ALU.mult)
    nc.vector.tensor_scalar(col02[:], col[:], 0.2, None, ALU.mult)
    # out = col*lrelu(h) = relu(0.8*col*h) + 0.2*col*h   (col > 0)
    # pairs interleave ACT relu and DVE combine for cross-engine overlap
    r1_all = big_p.tile([128, NB, 128], FP32, tag="r1_all")
    for g in range(NB // 2):
        for t in (2 * g, 2 * g + 1):
            nc.scalar.activation(
                r1_all[:, t, :], h_sb[:, t, :], AFT.Relu,
                scale=col08[:, t : t + 1],
            )
        for t in (2 * g, 2 * g + 1):
            o_sb = outsb_p.tile([128, 128], FP32, tag="o_sb")
            nc.vector.scalar_tensor_tensor(
                o_sb[:], h_sb[:, t, :], col02[:, t : t + 1], r1_all[:, t, :],
                ALU.mult, ALU.add,
            )
            (nc.sync if t % 2 == 0 else nc.gpsimd).dma_start(
                out_view[:, t, :], o_sb[:]
            )


def build_nc(num_devices: int = 8) -> "bass.Bass":
    nc = bacc.Bacc(
        "TRN2", target_bir_lowering=False, debug=False, num_devices=num_devices
    )
    x_d = nc.dram_tensor("x", [N, F], FP32, kind="ExternalInput")
    W_d = nc.dram_tensor("W", [F, F], FP32, kind="ExternalInput")
    wm_d = nc.dram_tensor("w_mlp", [F], FP32, kind="ExternalInput")
    bm_d = nc.dram_tensor("b_mlp", [1], FP32, kind="ExternalInput")
    iota_d = nc.dram_tensor("iota", [128], FP32, kind="ExternalInput")
    c42_d = nc.dram_tensor("c42", [4, 2], FP32, kind="ExternalInput")
    out_d = nc.dram_tensor("out", [N, F], FP32, kind="ExternalOutput")
    with tile.TileContext(nc) as tc:
        with ExitStack() as ctx:
            gat_kernel(
                ctx, tc, out_d.ap(), x_d.ap(), W_d.ap(), wm_d.ap(), bm_d.ap(),
                iota_d.ap(), c42_d.ap(),
            )
    nc.compile()
    return nc


_NC_CACHE: dict = {}


def run(x, W, w_mlp, b_mlp, trace=False, **spmd_kwargs):
    x = np.asarray(x, dtype=np.float32)
    W = np.asarray(W, dtype=np.float32)
    w_mlp = np.asarray(w_mlp, dtype=np.float32)
    b_mlp = np.asarray(b_mlp, dtype=np.float32)

    if "nc" not in _NC_CACHE:
        _NC_CACHE["nc"] = build_nc(num_devices=B)
    nc = _NC_CACHE["nc"]

    iota = np.arange(128, dtype=np.float32)
    c42 = np.array([[1, 0], [1, 0], [0, 1], [0, 1]], dtype=np.float32)
    in_maps = [
        {
            "x": np.ascontiguousarray(x[b, 0]),
            "W": W,
            "w_mlp": w_mlp,
            "b_mlp": b_mlp,
            "iota": iota,
            "c42": c42,
        }
        for b in range(B)
    ]
    res = run_bass_kernel_spmd(
        nc, in_maps, core_ids=list(range(B)), trace=trace, **spmd_kwargs
    )
    out = np.stack([res.results[b]["out"] for b in range(B)])[:, None]
    return out.astype(np.float32), res


def kernel(x, W, w_mlp, b_mlp):
    out, _ = run(x, W, w_mlp, b_mlp)
    return out


# revision 4
# speedup vs baseline: 1.0472x; 1.0339x over previous
"""GAT layer kernel for Trainium2 (Bass/Tile), data-parallel over batch on 8 cores.

v9: histogram-gather reformulation — never materializes the [N,N] mask.

Per-core math (batch item b, N=2048, F=128, K=128 grid):
    s = x @ (W @ w_mlp) + b;  p = exp(s);  q = exp(0.2 s)
    grid t_k = -m + k*(2m/K), m = 1.01*max|s|
    O1[j,k] = [s_j > t_k]      (N x K comparisons)
    O2[k,i] = [s_i <= -t_k]    (K x N comparisons)
    F_w[k]  = sum_j O1[j,k] w_j            (tiny matmul, w in {p,q,r,u})
    (Mw)_i  ~= sum_k O2[k,i] (F_w[k]-F_w[k-1])   (gather matmul)
    D = p*(Mp) + q*(Qtot-(Mq));  r = p/D; u = q/D
    col = p*(Mr) + q*(Utot-(Mu));  out = lrelu(h) * col,  h = x @ W
Threshold quantization flips mask entries only in a |s_i+s_j| < delta band
where exp(lrelu(.)) is branch-insensitive, so the error is O(delta^2).

Phase layout minimizes cross-engine round trips: all PE transposes first
(casts chase on DVE/ACT), then s-matmuls (s ready early), h-matmuls fill
the PE while DVE builds the grid + O1/O2 comparisons.
"""

import sys

if "/opt/trn_rl_repo" not in sys.path:
    sys.path.insert(0, "/opt/trn_rl_repo")

from contextlib import ExitStack

import numpy as np

import concourse.bass as bass
import concourse.bass_isa as bass_isa
import concourse.mybir as mybir
import concourse.tile as tile
from concourse import bacc
from concourse import masks
from concourse.bass_utils import run_bass_kernel_spmd

B, N, F = 8, 2048, 128
NB = N // 128  # 16 token blocks
K = 128        # threshold grid size
NEG_SLOPE = 0.2
FP32 = mybir.dt.float32
BF16 = mybir.dt.bfloat16
ALU = mybir.AluOpType
AFT = mybir.ActivationFunctionType


def gat_kernel(ctx, tc, out_d, x_d, W_d, wm_d, bm_d, iota_d, c42_d):
    nc = tc.nc

    const_p = ctx.enter_context(tc.tile_pool(name="const", bufs=1))
    big_p = ctx.enter_context(tc.tile_pool(name="big", bufs=1))
    vec_p = ctx.enter_context(tc.tile_pool(name="vec", bufs=1))
    outsb_p = ctx.enter_context(tc.tile_pool(name="outsb", bufs=6))
    # PSUM: 8 banks. big=4 (h_ps, held to the end), tr=3 rotating, sm=1.
    ps_big = ctx.enter_context(tc.tile_pool(name="ps_big", bufs=1, space="PSUM"))
    ps_tr = ctx.enter_context(tc.tile_pool(name="ps_tr", bufs=3, space="PSUM"))
    ps_sm = ctx.enter_context(tc.tile_pool(name="ps_sm", bufs=1, space="PSUM"))

    # ---------------- input DMAs + gpsimd consts first ------------------
    W_sb = const_p.tile([128, 128], FP32, tag="W_sb")
    nc.sync.dma_start(W_sb[:], W_d[:, :])
    ident_f = const_p.tile([128, 128], FP32, tag="ident_f")
    ident_b = const_p.tile([128, 128], BF16, tag="ident_b")
    masks.make_identity(nc, ident_f[:])
    masks.make_identity(nc, ident_b[:])
    ones_f = const_p.tile([128, 1], FP32, tag="ones_f")
    nc.gpsimd.memset(ones_f[:], 1.0)
    ones_row_f = const_p.tile([1, 128], FP32, tag="ones_row_f")
    nc.gpsimd.memset(ones_row_f[:], 1.0)
    ones_row_b = const_p.tile([1, 128], BF16, tag="ones_row_b")
    nc.gpsimd.memset(ones_row_b[:], 1.0)
    neg_ones_row_f = const_p.tile([1, 128], FP32, tag="neg_ones_row_f")
    nc.gpsimd.memset(neg_ones_row_f[:], -1.0)

    x_view = x_d.rearrange("(t p) f -> p t f", p=128)
    x_sb = big_p.tile([128, NB, 128], FP32, tag="x_sb")
    x_q_of = {2: nc.scalar, 8: nc.scalar, 14: nc.scalar}
    x_alt = [nc.sync, nc.gpsimd]
    n_alt = 0
    for t in range(NB):
        q = x_q_of.get(t)
        if q is None:
            q = x_alt[n_alt % 2]
            n_alt += 1
        q.dma_start(x_sb[:, t, :], x_view[:, t, :])
    wm_sb = const_p.tile([128, 1], FP32, tag="wm_sb")
    nc.scalar.dma_start(wm_sb[:], wm_d.rearrange("(p o) -> p o", o=1))
    b_sb = const_p.tile([1, 1], FP32, tag="b_sb")
    nc.scalar.dma_start(b_sb[:], bm_d.rearrange("(p o) -> p o", o=1))
    iota_col = const_p.tile([128, 1], FP32, tag="iota_col")
    nc.scalar.dma_start(iota_col[:], iota_d.rearrange("(p o) -> p o", o=1))
    iota_row = const_p.tile([1, 128], FP32, tag="iota_row")
    nc.scalar.dma_start(iota_row[:], iota_d.rearrange("(o k) -> o k", o=1))
    C42 = const_p.tile([4, 2], FP32, tag="C42")
    nc.scalar.dma_start(C42[:], c42_d[:, :])

    # ACT tables (Exp for p/q, Relu for the output stage)
    warm = const_p.tile([128, 1], FP32, tag="warm")
    nc.scalar.activation(warm[:], ones_f[:], AFT.Exp)
    nc.scalar.activation(warm[:], ones_f[:], AFT.Relu)

    # b broadcast to [128,1] via K=1 PE matmul
    b_ps = ps_sm.tile([128, 1], FP32, tag="sm")
    nc.tensor.matmul(b_ps[:], lhsT=ones_row_f[:], rhs=b_sb[:], start=True, stop=True)
    b_bc = const_p.tile([128, 1], FP32, tag="b_bc")
    nc.vector.tensor_copy(b_bc[:], b_ps[:])

    # ---------------- W -> bf16; v = W @ w_mlp; vk pair ------------------
    W_hi = const_p.tile([128, 128], BF16, tag="W_hi")
    nc.scalar.copy(W_hi[:], W_sb[:])
    WT_ps = ps_sm.tile([128, 128], FP32, tag="sm")
    nc.tensor.transpose(WT_ps[:], W_sb[:], ident_f[:])
    WT_sb = vec_p.tile([128, 128], FP32, tag="WT_sb")
    nc.vector.tensor_copy(WT_sb[:], WT_ps[:])
    v_ps = ps_sm.tile([128, 1], FP32, tag="sm")
    nc.tensor.matmul(v_ps[:], lhsT=WT_sb[:], rhs=wm_sb[:], start=True, stop=True)
    v_sb = vec_p.tile([128, 1], FP32, tag="v_sb")
    nc.vector.tensor_copy(v_sb[:], v_ps[:])
    vk = vec_p.tile([128, 2], BF16, tag="vk")
    nc.vector.tensor_copy(vk[:, 0:1], v_sb[:])
    v_hi32 = vec_p.tile([128, 1], FP32, tag="v_hi32")
    nc.vector.tensor_copy(v_hi32[:], vk[:, 0:1])
    nc.vector.tensor_tensor(vk[:, 1:2], v_sb[:], v_hi32[:], ALU.subtract)

    # ---------------- phase A: all transposes, casts chase ---------------
    xT_hi = big_p.tile([128, N], BF16, tag="xT_hi")  # [f, tok]
    for t in range(NB):
        sl = slice(t * 128, (t + 1) * 128)
        tr32 = ps_tr.tile([128, 128], FP32, tag="trb")
        nc.tensor.transpose(tr32[:], x_sb[:, t, :], ident_f[:])
        if t % 2 == 0:
            nc.vector.tensor_copy(xT_hi[:, sl], tr32[:])
        else:
            nc.scalar.copy(xT_hi[:, sl], tr32[:])

    # ---------------- phase B: s matmuls, s assembly ---------------------
    s4_ps = ps_sm.tile([128, NB, 2], FP32, tag="sm")
    for t in range(NB):
        sl = slice(t * 128, (t + 1) * 128)
        nc.tensor.matmul(
            s4_ps[:, t, :], lhsT=xT_hi[:, sl], rhs=vk[:], start=True, stop=True
        )
    s4_sb = vec_p.tile([128, NB, 2], FP32, tag="s4_sb")
    nc.vector.tensor_copy(s4_sb[:], s4_ps[:])
    s12 = vec_p.tile([128, NB], FP32, tag="s12")
    nc.vector.tensor_tensor(s12[:], s4_sb[:, :, 0], s4_sb[:, :, 1], ALU.add)
    s_mat = vec_p.tile([128, NB], FP32, tag="s_mat")
    nc.vector.tensor_scalar(s_mat[:], s12[:], b_bc[:, 0:1], None, ALU.add)
    s_hi = vec_p.tile([128, NB], BF16, tag="s_hi")
    nc.vector.tensor_copy(s_hi[:], s_mat[:])
    sT_ps = ps_tr.tile([16, 128], BF16, tag="trb")
    nc.tensor.transpose(sT_ps[:], s_hi[:], ident_b[:])
    sT_sb = vec_p.tile([16, 128], BF16, tag="sT_sb")
    nc.vector.tensor_copy(sT_sb[:], sT_ps[:])
    s_flat = vec_p.tile([1, N], BF16, tag="s_flat")
    nc.sync.dma_start(s_flat[0:1, :], sT_sb[:, :])

    # grid: m = 1.01*max|s|, thresholds along free (Tgrid) and partitions
    amax = vec_p.tile([128, 1], FP32, tag="amax")
    nc.vector.tensor_reduce(
        amax[:], s_mat[:], axis=mybir.AxisListType.X, op=ALU.max,
        apply_absolute_value=True,
    )
    amaxT_ps = ps_tr.tile([1, 128], FP32, tag="trb")
    nc.tensor.transpose(amaxT_ps[:], amax[:], ident_f[:])
    # exps + Pk splits fill the DVE while the partition-max round-trips PE
    p_v = vec_p.tile([128, NB], FP32, tag="p_v")
    nc.scalar.activation(p_v[:], s_mat[:], AFT.Exp)
    q_v = vec_p.tile([128, NB], FP32, tag="q_v")
    nc.scalar.activation(q_v[:], s_mat[:], AFT.Exp, scale=NEG_SLOPE)
    Pk = vec_p.tile([128, NB, 4], BF16, tag="Pk")
    nc.vector.tensor_copy(Pk[:, :, 0], p_v[:])
    p_hi32 = vec_p.tile([128, NB], FP32, tag="p_hi32")
    nc.vector.tensor_copy(p_hi32[:], Pk[:, :, 0])
    nc.vector.tensor_tensor(Pk[:, :, 1], p_v[:], p_hi32[:], ALU.subtract)
    nc.vector.tensor_copy(Pk[:, :, 2], q_v[:])
    q_hi32 = vec_p.tile([128, NB], FP32, tag="q_hi32")
    nc.vector.tensor_copy(q_hi32[:], Pk[:, :, 2])
    nc.vector.tensor_tensor(Pk[:, :, 3], q_v[:], q_hi32[:], ALU.subtract)
    amaxT = vec_p.tile([1, 128], FP32, tag="amaxT")
    nc.vector.tensor_copy(amaxT[:], amaxT_ps[:])
    m_sc = vec_p.tile([1, 1], FP32, tag="m_sc")
    nc.vector.tensor_reduce(m_sc[:], amaxT[:], axis=mybir.AxisListType.X, op=ALU.max)
    m_ps = ps_sm.tile([128, 1], FP32, tag="sm")
    nc.tensor.matmul(m_ps[:], lhsT=ones_row_f[:], rhs=m_sc[:], start=True, stop=True)
    m_bc = vec_p.tile([128, 1], FP32, tag="m_bc")
    nc.vector.tensor_copy(m_bc[:], m_ps[:])
    m_col = vec_p.tile([128, 1], FP32, tag="m_col")
    nc.vector.tensor_scalar(m_col[:], m_bc[:], 1.01, None, ALU.mult)
    negm_col = vec_p.tile([128, 1], FP32, tag="negm_col")
    nc.vector.tensor_scalar(negm_col[:], m_col[:], -1.0, None, ALU.mult)
    delta_col = vec_p.tile([128, 1], FP32, tag="delta_col")
    nc.vector.tensor_scalar(delta_col[:], m_col[:], 2.0 / K, None, ALU.mult)
    negdelta_col = vec_p.tile([128, 1], FP32, tag="negdelta_col")
    nc.vector.tensor_scalar(negdelta_col[:], m_col[:], -2.0 / K, None, ALU.mult)
    negt_col = vec_p.tile([128, 1], FP32, tag="negt_col")
    nc.vector.tensor_scalar(
        negt_col[:], iota_col[:], negdelta_col[:, 0:1], m_col[:, 0:1],
        ALU.mult, ALU.add,
    )
    iota_bc_ps = ps_tr.tile([128, 128], FP32, tag="trb")
    nc.tensor.matmul(
        iota_bc_ps[:], lhsT=ones_row_f[:], rhs=iota_row[:], start=True, stop=True
    )
    Tgrid_row = vec_p.tile([128, K], FP32, tag="Tgrid_row")
    nc.vector.tensor_scalar(
        Tgrid_row[:], iota_bc_ps[:], delta_col[:, 0:1], negm_col[:, 0:1],
        ALU.mult, ALU.add,
    )


    # ---------------- phase C: S broadcasts (to SBUF), O1 + F1 -----------
    O1 = big_p.tile([128, NB, K], BF16, tag="O1")
    O2 = big_p.tile([128, NB, 128], BF16, tag="O2")
    S_row = big_p.tile([128, NB, 128], BF16, tag="S_row")
    for c in range(4):
        S_ps = ps_tr.tile([128, 512], FP32, tag="trb")
        nc.tensor.matmul(
            S_ps[:], lhsT=ones_row_b[:], rhs=s_flat[0:1, c * 512 : (c + 1) * 512],
            start=True, stop=True,
        )
        nc.scalar.copy(S_row[:, 4 * c : 4 * (c + 1), :], S_ps[:])
    F1_ps = ps_sm.tile([4, K], FP32, tag="sm")
    for a in range(NB):
        nc.vector.tensor_scalar(
            O1[:, a, :], Tgrid_row[:], s_mat[:, a : a + 1], None, ALU.is_lt
        )
        nc.tensor.matmul(
            F1_ps[:], lhsT=Pk[:, a, :], rhs=O1[:, a, :],
            start=(a == 0), stop=(a == NB - 1),
        )

    # h matmuls fill the PE behind the F1 sprint; h copied out of PSUM so
    # the output stage reads SBUF from both engines without serializing
    h_ps = ps_big.tile([128, NB, 128], FP32, tag="bigps")
    for t in range(NB):
        sl = slice(t * 128, (t + 1) * 128)
        nc.tensor.matmul(
            h_ps[:, t, :], lhsT=xT_hi[:, sl], rhs=W_hi[:], start=True, stop=True
        )
    h_sb = big_p.tile([128, NB, 128], FP32, tag="h_sb")
    for i2 in range(NB // 2):
        nc.scalar.copy(h_sb[:, 2 * i2 : 2 * i2 + 2, :], h_ps[:, 2 * i2 : 2 * i2 + 2, :])

    # Qtot / Ptot sums and broadcasts (needed at D assembly)
    qs = vec_p.tile([128, 1], FP32, tag="qs")
    nc.vector.reduce_sum(qs[:], q_v[:], axis=mybir.AxisListType.X)
    Qt_ps = ps_sm.tile([1, 1], FP32, tag="sm")
    nc.tensor.matmul(Qt_ps[:], lhsT=qs[:], rhs=ones_f[:, 0:1], start=True, stop=True)
    Qt_sb = vec_p.tile([1, 1], FP32, tag="Qt_sb")
    nc.vector.tensor_copy(Qt_sb[:], Qt_ps[:])
    nQt_ps = ps_sm.tile([128, 1], FP32, tag="sm")
    nc.tensor.matmul(nQt_ps[:], lhsT=neg_ones_row_f[:], rhs=Qt_sb[:], start=True, stop=True)
    negQt_col = vec_p.tile([128, 1], FP32, tag="negQt_col")
    nc.vector.tensor_copy(negQt_col[:], nQt_ps[:])

    # ---------------- D1 prep: diffs + transpose-combine -----------------
    def d_prep(F_ps, nm):
        F_sb = vec_p.tile([4, K], FP32, tag=nm + "F_sb")
        nc.vector.tensor_copy(F_sb[:], F_ps[:])
        D1_row = vec_p.tile([4, K], FP32, tag=nm + "D1_row")
        nc.vector.tensor_copy(D1_row[:, 0:1], F_sb[:, 0:1])
        nc.vector.tensor_tensor(
            D1_row[:, 1:K], F_sb[:, 1:K], F_sb[:, 0 : K - 1], ALU.subtract
        )
        Dcol_ps = ps_sm.tile([128, 2], FP32, tag="sm")
        nc.tensor.matmul(Dcol_ps[:], lhsT=D1_row[:], rhs=C42[:], start=True, stop=True)
        Dcol = vec_p.tile([128, 2], FP32, tag=nm + "Dcol")
        nc.vector.tensor_copy(Dcol[:], Dcol_ps[:])
        Dk = vec_p.tile([128, 4], BF16, tag=nm + "Dk")
        nc.vector.tensor_copy(Dk[:, 0:1], Dcol[:, 0:1])
        h32 = vec_p.tile([128, 2], FP32, tag=nm + "h32")
        nc.vector.tensor_copy(h32[:, 0:1], Dk[:, 0:1])
        nc.vector.tensor_tensor(Dk[:, 1:2], Dcol[:, 0:1], h32[:, 0:1], ALU.subtract)
        nc.vector.tensor_copy(Dk[:, 2:3], Dcol[:, 1:2])
        nc.vector.tensor_copy(h32[:, 1:2], Dk[:, 2:3])
        nc.vector.tensor_tensor(Dk[:, 3:4], Dcol[:, 1:2], h32[:, 1:2], ALU.subtract)
        return Dk

    Dk1 = d_prep(F1_ps, "g1")

    # ------------- O2 comparisons interleaved with gather 1 --------------
    MPq_ps = ps_sm.tile([128, NB, 4], FP32, tag="sm")
    for c in range(4):
        nc.vector.tensor_scalar(
            O2[:, c * 4 : (c + 1) * 4, :],
            S_row[:, 4 * c : 4 * (c + 1), :],
            negt_col[:, 0:1], None, ALU.is_le,
        )
        for t in range(4 * c, 4 * c + 4):
            nc.tensor.matmul(
                MPq_ps[:, t, :], lhsT=O2[:, t, :], rhs=Dk1[:], start=True, stop=True
            )
    Dp = vec_p.tile([128, NB, 4], FP32, tag="Dp")
    # ------------- D assembly in halves; F2 starts after half 1 ----------
    MP = vec_p.tile([128, NB], FP32, tag="MP")
    MQ = vec_p.tile([128, NB], FP32, tag="MQ")
    t2 = vec_p.tile([128, NB], FP32, tag="t2")
    t3 = vec_p.tile([128, NB], FP32, tag="t3")
    D_v = vec_p.tile([128, NB], FP32, tag="D_v")
    invD = vec_p.tile([128, NB], FP32, tag="invD")
    r_v = vec_p.tile([128, NB], FP32, tag="r_v")
    u_v = vec_p.tile([128, NB], FP32, tag="u_v")
    Rk = vec_p.tile([128, NB, 4], BF16, tag="Rk")
    r_hi32 = vec_p.tile([128, NB], FP32, tag="r_hi32")
    u_hi32 = vec_p.tile([128, NB], FP32, tag="u_hi32")
    F2_ps = ps_sm.tile([4, K], FP32, tag="sm")
    H = NB // 2
    for half in range(2):
        hs = slice(half * H, (half + 1) * H)
        nc.vector.tensor_copy(Dp[:, hs, :], MPq_ps[:, hs, :])
        nc.vector.tensor_tensor(MP[:, hs], Dp[:, hs, 0], Dp[:, hs, 1], ALU.add)
        nc.vector.tensor_tensor(MQ[:, hs], Dp[:, hs, 2], Dp[:, hs, 3], ALU.add)
        nc.vector.scalar_tensor_tensor(
            t2[:, hs], MQ[:, hs], negQt_col[:, 0:1], q_v[:, hs], ALU.add, ALU.mult
        )
        nc.vector.tensor_tensor(t3[:, hs], p_v[:, hs], MP[:, hs], ALU.mult)
        nc.vector.tensor_tensor(D_v[:, hs], t3[:, hs], t2[:, hs], ALU.subtract)
        nc.vector.reciprocal(invD[:, hs], D_v[:, hs])
        nc.vector.tensor_tensor(r_v[:, hs], p_v[:, hs], invD[:, hs], ALU.mult)
        nc.vector.tensor_tensor(u_v[:, hs], q_v[:, hs], invD[:, hs], ALU.mult)
        nc.vector.tensor_copy(Rk[:, hs, 0], r_v[:, hs])
        nc.vector.tensor_copy(r_hi32[:, hs], Rk[:, hs, 0])
        nc.vector.tensor_tensor(Rk[:, hs, 1], r_v[:, hs], r_hi32[:, hs], ALU.subtract)
        nc.vector.tensor_copy(Rk[:, hs, 2], u_v[:, hs])
        nc.vector.tensor_copy(u_hi32[:, hs], Rk[:, hs, 2])
        nc.vector.tensor_tensor(Rk[:, hs, 3], u_v[:, hs], u_hi32[:, hs], ALU.subtract)
        for a in range(half * H, (half + 1) * H):
            nc.tensor.matmul(
                F2_ps[:], lhsT=Rk[:, a, :], rhs=O1[:, a, :],
                start=(a == 0), stop=(a == NB - 1),
            )
    us = vec_p.tile([128, 1], FP32, tag="us")
    nc.vector.reduce_sum(us[:], u_v[:], axis=mybir.AxisListType.X)

    # Utot scalar -> -Utot broadcast
    Ut_ps = ps_sm.tile([1, 1], FP32, tag="sm")
    nc.tensor.matmul(Ut_ps[:], lhsT=us[:], rhs=ones_f[:, 0:1], start=True, stop=True)
    Ut_sb = vec_p.tile([1, 1], FP32, tag="Ut_sb")
    nc.vector.tensor_copy(Ut_sb[:], Ut_ps[:])
    nUt_ps = ps_sm.tile([128, 1], FP32, tag="sm")
    nc.tensor.matmul(nUt_ps[:], lhsT=neg_ones_row_f[:], rhs=Ut_sb[:], start=True, stop=True)
    negUt_col = vec_p.tile([128, 1], FP32, tag="negUt_col")
    nc.vector.tensor_copy(negUt_col[:], nUt_ps[:])

    Dk2 = d_prep(F2_ps, "g2")

    # ---------------- gather 2 + output ----------------------------------
    out_view = out_d.rearrange("(t p) f -> p t f", p=128)
    Gp_ps = ps_sm.tile([128, NB, 4], FP32, tag="sm")
    Gp = vec_p.tile([128, NB, 4], FP32, tag="Gp")
    for t in range(NB):
        nc.tensor.matmul(
            Gp_ps[:, t, :], lhsT=O2[:, t, :], rhs=Dk2[:], start=True, stop=True
        )
    nc.scalar.copy(Gp[:], Gp_ps[:])
    MR = vec_p.tile([128, NB], FP32, tag="MR")
    MU = vec_p.tile([128, NB], FP32, tag="MU")
    z2 = vec_p.tile([128, NB], FP32, tag="z2")
    z3 = vec_p.tile([128, NB], FP32, tag="z3")
    col = vec_p.tile([128, NB], FP32, tag="col")
    col08 = vec_p.tile([128, NB], FP32, tag="col08")
    col02 = vec_p.tile([128, NB], FP32, tag="col02")
    nc.vector.tensor_tensor(MR[:], Gp[:, :, 0], Gp[:, :, 1], ALU.add)
    nc.vector.tensor_tensor(MU[:], Gp[:, :, 2], Gp[:, :, 3], ALU.add)
    nc.vector.scalar_tensor_tensor(
        z2[:], MU[:], negUt_col[:, 0:1], q_v[:], ALU.add, ALU.mult
    )
    nc.vector.tensor_tensor(z3[:], p_v[:], MR[:], ALU.mult)
    nc.vector.tensor_tensor(col[:], z3[:], z2[:], ALU.subtract)
    nc.vector.tensor_scalar(col08[:], col[:], 0.8, None, ALU.mult)
    nc.vector.tensor_scalar(col02[:], col[:], 0.2, None, ALU.mult)
    # out = col*lrelu(h) = relu(0.8*col*h) + 0.2*col*h   (col > 0)
    # pairs interleave ACT relu and DVE combine for cross-engine overlap
    r1_all = big_p.tile([128, NB, 128], FP32, tag="r1_all")
    for g in range(NB // 2):
        for t in (2 * g, 2 * g + 1):
            nc.scalar.activation(
                r1_all[:, t, :], h_sb[:, t, :], AFT.Relu,
                scale=col08[:, t : t + 1],
            )
        for t in (2 * g, 2 * g + 1):
            o_sb = outsb_p.tile([128, 128], FP32, tag="o_sb")
            nc.vector.scalar_tensor_tensor(
                o_sb[:], h_sb[:, t, :], col02[:, t : t + 1], r1_all[:, t, :],
                ALU.mult, ALU.add,
            )
            (nc.sync if t % 2 == 0 else nc.gpsimd).dma_start(
                out_view[:, t, :], o_sb[:]
            )


def build_nc(num_devices: int = 8) -> "bass.Bass":
    nc = bacc.Bacc(
        "TRN2", target_bir_lowering=False, debug=False, num_devices=num_devices
    )
    x_d = nc.dram_tensor("x", [N, F], FP32, kind="ExternalInput")
    W_d = nc.dram_tensor("W", [F, F], FP32, kind="ExternalInput")
    wm_d = nc.dram_tensor("w_mlp", [F], FP32, kind="ExternalInput")
    bm_d = nc.dram_tensor("b_mlp", [1], FP32, kind="ExternalInput")
    iota_d = nc.dram_tensor("iota", [128], FP32, kind="ExternalInput")
    c42_d = nc.dram_tensor("c42", [4, 2], FP32, kind="ExternalInput")
    out_d = nc.dram_tensor("out", [N, F], FP32, kind="ExternalOutput")
    with tile.TileContext(nc) as tc:
        with ExitStack() as ctx:
            gat_kernel(
                ctx, tc, out_d.ap(), x_d.ap(), W_d.ap(), wm_d.ap(), bm_d.ap(),
                iota_d.ap(), c42_d.ap(),
            )
    nc.compile()
    return nc


_NC_CACHE: dict = {}


def run(x, W, w_mlp, b_mlp, trace=False, **spmd_kwargs):
    x = np.asarray(x, dtype=np.float32)
    W = np.asarray(W, dtype=np.float32)
    w_mlp = np.asarray(w_mlp, dtype=np.float32)
    b_mlp = np.asarray(b_mlp, dtype=np.float32)

    if "nc" not in _NC_CACHE:
        _NC_CACHE["nc"] = build_nc(num_devices=B)
    nc = _NC_CACHE["nc"]

    iota = np.arange(128, dtype=np.float32)
    c42 = np.array([[1, 0], [1, 0], [0, 1], [0, 1]], dtype=np.float32)
    in_maps = [
        {
            "x": np.ascontiguousarray(x[b, 0]),
            "W": W,
            "w_mlp": w_mlp,
            "b_mlp": b_mlp,
            "iota": iota,
            "c42": c42,
        }
        for b in range(B)
    ]
    res = run_bass_kernel_spmd(
        nc, in_maps, core_ids=list(range(B)), trace=trace, **spmd_kwargs
    )
    out = np.stack([res.results[b]["out"] for b in range(B)])[:, None]
    return out.astype(np.float32), res


def kernel(x, W, w_mlp, b_mlp):
    out, _ = run(x, W, w_mlp, b_mlp)
    return out


# revision 5
# speedup vs baseline: 1.0613x; 1.0134x over previous
"""GAT layer kernel for Trainium2 (Bass/Tile), data-parallel over batch on 8 cores.

v9: histogram-gather reformulation — never materializes the [N,N] mask.

Per-core math (batch item b, N=2048, F=128, K=128 grid):
    s = x @ (W @ w_mlp) + b;  p = exp(s);  q = exp(0.2 s)
    grid t_k = -m + k*(2m/K), m = 1.01*max|s|
    O1[j,k] = [s_j > t_k]      (N x K comparisons)
    O2[k,i] = [s_i <= -t_k]    (K x N comparisons)
    F_w[k]  = sum_j O1[j,k] w_j            (tiny matmul, w in {p,q,r,u})
    (Mw)_i  ~= sum_k O2[k,i] (F_w[k]-F_w[k-1])   (gather matmul)
    D = p*(Mp) + q*(Qtot-(Mq));  r = p/D; u = q/D
    col = p*(Mr) + q*(Utot-(Mu));  out = lrelu(h) * col,  h = x @ W
Threshold quantization flips mask entries only in a |s_i+s_j| < delta band
where exp(lrelu(.)) is branch-insensitive, so the error is O(delta^2).

Phase layout minimizes cross-engine round trips: all PE transposes first
(casts chase on DVE/ACT), then s-matmuls (s ready early), h-matmuls fill
the PE while DVE builds the grid + O1/O2 comparisons.
"""

import sys

if "/opt/trn_rl_repo" not in sys.path:
    sys.path.insert(0, "/opt/trn_rl_repo")

from contextlib import ExitStack

import numpy as np

import concourse.bass as bass
import concourse.bass_isa as bass_isa
import concourse.mybir as mybir
import concourse.tile as tile
from concourse import bacc
from concourse import masks
from concourse.bass_utils import run_bass_kernel_spmd

B, N, F = 8, 2048, 128
NB = N // 128  # 16 token blocks
K = 32         # threshold grid size
NEG_SLOPE = 0.2
FP32 = mybir.dt.float32
BF16 = mybir.dt.bfloat16
ALU = mybir.AluOpType
AFT = mybir.ActivationFunctionType


def gat_kernel(ctx, tc, out_d, x_d, W_d, wm_d, bm_d, iota_d, c42_d):
    nc = tc.nc

    const_p = ctx.enter_context(tc.tile_pool(name="const", bufs=1))
    big_p = ctx.enter_context(tc.tile_pool(name="big", bufs=1))
    vec_p = ctx.enter_context(tc.tile_pool(name="vec", bufs=1))
    outsb_p = ctx.enter_context(tc.tile_pool(name="outsb", bufs=6))
    # PSUM: 8 banks. big=4 (h_ps, held to the end), tr=3 rotating, sm=1.
    ps_big = ctx.enter_context(tc.tile_pool(name="ps_big", bufs=1, space="PSUM"))
    ps_tr = ctx.enter_context(tc.tile_pool(name="ps_tr", bufs=3, space="PSUM"))
    ps_sm = ctx.enter_context(tc.tile_pool(name="ps_sm", bufs=1, space="PSUM"))

    # ---------------- input DMAs + gpsimd consts first ------------------
    W_sb = const_p.tile([128, 128], FP32, tag="W_sb")
    nc.sync.dma_start(W_sb[:], W_d[:, :])
    ident_f = const_p.tile([128, 128], FP32, tag="ident_f")
    ident_b = const_p.tile([128, 128], BF16, tag="ident_b")
    masks.make_identity(nc, ident_f[:])
    masks.make_identity(nc, ident_b[:])
    ones_f = const_p.tile([128, 1], FP32, tag="ones_f")
    nc.gpsimd.memset(ones_f[:], 1.0)
    ones_row_f = const_p.tile([1, 128], FP32, tag="ones_row_f")
    nc.gpsimd.memset(ones_row_f[:], 1.0)
    ones_row_b = const_p.tile([1, 128], BF16, tag="ones_row_b")
    nc.gpsimd.memset(ones_row_b[:], 1.0)
    neg_ones_row_f = const_p.tile([1, 128], FP32, tag="neg_ones_row_f")
    nc.gpsimd.memset(neg_ones_row_f[:], -1.0)

    x_view = x_d.rearrange("(t p) f -> p t f", p=128)
    x_sb = big_p.tile([128, NB, 128], FP32, tag="x_sb")
    x_q_of = {2: nc.scalar, 8: nc.scalar, 14: nc.scalar}
    x_alt = [nc.sync, nc.gpsimd]
    n_alt = 0
    for t in range(NB):
        q = x_q_of.get(t)
        if q is None:
            q = x_alt[n_alt % 2]
            n_alt += 1
        q.dma_start(x_sb[:, t, :], x_view[:, t, :])
    wm_sb = const_p.tile([128, 1], FP32, tag="wm_sb")
    nc.scalar.dma_start(wm_sb[:], wm_d.rearrange("(p o) -> p o", o=1))
    b_sb = const_p.tile([1, 1], FP32, tag="b_sb")
    nc.scalar.dma_start(b_sb[:], bm_d.rearrange("(p o) -> p o", o=1))
    iota_col = const_p.tile([128, 1], FP32, tag="iota_col")
    nc.scalar.dma_start(iota_col[:], iota_d.rearrange("(p o) -> p o", o=1))
    iota_row = const_p.tile([1, 128], FP32, tag="iota_row")
    nc.scalar.dma_start(iota_row[:], iota_d.rearrange("(o k) -> o k", o=1))
    C42 = const_p.tile([4, 2], FP32, tag="C42")
    nc.scalar.dma_start(C42[:], c42_d[:, :])

    # ACT tables (Exp for p/q, Relu for the output stage)
    warm = const_p.tile([128, 1], FP32, tag="warm")
    nc.scalar.activation(warm[:], ones_f[:], AFT.Exp)
    nc.scalar.activation(warm[:], ones_f[:], AFT.Relu)

    # b broadcast to [128,1] via K=1 PE matmul
    b_ps = ps_sm.tile([128, 1], FP32, tag="sm")
    nc.tensor.matmul(b_ps[:], lhsT=ones_row_f[:], rhs=b_sb[:], start=True, stop=True)
    b_bc = const_p.tile([128, 1], FP32, tag="b_bc")
    nc.vector.tensor_copy(b_bc[:], b_ps[:])

    # ---------------- W -> bf16; v = W @ w_mlp; vk pair ------------------
    W_hi = const_p.tile([128, 128], BF16, tag="W_hi")
    nc.scalar.copy(W_hi[:], W_sb[:])
    WT_ps = ps_sm.tile([128, 128], FP32, tag="sm")
    nc.tensor.transpose(WT_ps[:], W_sb[:], ident_f[:])
    WT_sb = vec_p.tile([128, 128], FP32, tag="WT_sb")
    nc.vector.tensor_copy(WT_sb[:], WT_ps[:])
    v_ps = ps_sm.tile([128, 1], FP32, tag="sm")
    nc.tensor.matmul(v_ps[:], lhsT=WT_sb[:], rhs=wm_sb[:], start=True, stop=True)
    v_sb = vec_p.tile([128, 1], FP32, tag="v_sb")
    nc.vector.tensor_copy(v_sb[:], v_ps[:])
    vk = vec_p.tile([128, 2], BF16, tag="vk")
    nc.vector.tensor_copy(vk[:, 0:1], v_sb[:])
    v_hi32 = vec_p.tile([128, 1], FP32, tag="v_hi32")
    nc.vector.tensor_copy(v_hi32[:], vk[:, 0:1])
    nc.vector.tensor_tensor(vk[:, 1:2], v_sb[:], v_hi32[:], ALU.subtract)

    # ---------------- phase A: all transposes, casts chase ---------------
    xT_hi = big_p.tile([128, N], BF16, tag="xT_hi")  # [f, tok]
    for t in range(NB):
        sl = slice(t * 128, (t + 1) * 128)
        tr32 = ps_tr.tile([128, 128], FP32, tag="trb")
        nc.tensor.transpose(tr32[:], x_sb[:, t, :], ident_f[:])
        if t % 2 == 0:
            nc.vector.tensor_copy(xT_hi[:, sl], tr32[:])
        else:
            nc.scalar.copy(xT_hi[:, sl], tr32[:])

    # ---------------- phase B: s matmuls, s assembly ---------------------
    s4_ps = ps_sm.tile([128, NB, 2], FP32, tag="sm")
    for t in range(NB):
        sl = slice(t * 128, (t + 1) * 128)
        nc.tensor.matmul(
            s4_ps[:, t, :], lhsT=xT_hi[:, sl], rhs=vk[:], start=True, stop=True
        )
    s4_sb = vec_p.tile([128, NB, 2], FP32, tag="s4_sb")
    nc.vector.tensor_copy(s4_sb[:], s4_ps[:])
    s12 = vec_p.tile([128, NB], FP32, tag="s12")
    nc.vector.tensor_tensor(s12[:], s4_sb[:, :, 0], s4_sb[:, :, 1], ALU.add)
    s_mat = vec_p.tile([128, NB], FP32, tag="s_mat")
    nc.vector.tensor_scalar(s_mat[:], s12[:], b_bc[:, 0:1], None, ALU.add)
    s_hi = vec_p.tile([128, NB], BF16, tag="s_hi")
    nc.vector.tensor_copy(s_hi[:], s_mat[:])
    sT_ps = ps_tr.tile([16, 128], BF16, tag="trb")
    nc.tensor.transpose(sT_ps[:], s_hi[:], ident_b[:])
    sT_sb = vec_p.tile([16, 128], BF16, tag="sT_sb")
    nc.vector.tensor_copy(sT_sb[:], sT_ps[:])
    s_flat = vec_p.tile([1, N], BF16, tag="s_flat")
    nc.sync.dma_start(s_flat[0:1, :], sT_sb[:, :])

    # grid: m = 1.01*max|s|, thresholds along free (Tgrid) and partitions
    amax = vec_p.tile([128, 1], FP32, tag="amax")
    nc.vector.tensor_reduce(
        amax[:], s_mat[:], axis=mybir.AxisListType.X, op=ALU.max,
        apply_absolute_value=True,
    )
    amaxT_ps = ps_tr.tile([1, 128], FP32, tag="trb")
    nc.tensor.transpose(amaxT_ps[:], amax[:], ident_f[:])
    # exps + Pk splits fill the DVE while the partition-max round-trips PE
    p_v = vec_p.tile([128, NB], FP32, tag="p_v")
    nc.scalar.activation(p_v[:], s_mat[:], AFT.Exp)
    q_v = vec_p.tile([128, NB], FP32, tag="q_v")
    nc.scalar.activation(q_v[:], s_mat[:], AFT.Exp, scale=NEG_SLOPE)
    Pk = vec_p.tile([128, NB, 4], BF16, tag="Pk")
    nc.vector.tensor_copy(Pk[:, :, 0], p_v[:])
    p_hi32 = vec_p.tile([128, NB], FP32, tag="p_hi32")
    nc.vector.tensor_copy(p_hi32[:], Pk[:, :, 0])
    nc.vector.tensor_tensor(Pk[:, :, 1], p_v[:], p_hi32[:], ALU.subtract)
    nc.vector.tensor_copy(Pk[:, :, 2], q_v[:])
    q_hi32 = vec_p.tile([128, NB], FP32, tag="q_hi32")
    nc.vector.tensor_copy(q_hi32[:], Pk[:, :, 2])
    nc.vector.tensor_tensor(Pk[:, :, 3], q_v[:], q_hi32[:], ALU.subtract)
    amaxT = vec_p.tile([1, 128], FP32, tag="amaxT")
    nc.vector.tensor_copy(amaxT[:], amaxT_ps[:])
    m_sc = vec_p.tile([1, 1], FP32, tag="m_sc")
    nc.vector.tensor_reduce(m_sc[:], amaxT[:], axis=mybir.AxisListType.X, op=ALU.max)
    m_ps = ps_sm.tile([128, 1], FP32, tag="sm")
    nc.tensor.matmul(m_ps[:], lhsT=ones_row_f[:], rhs=m_sc[:], start=True, stop=True)
    m_bc = vec_p.tile([128, 1], FP32, tag="m_bc")
    nc.vector.tensor_copy(m_bc[:], m_ps[:])
    m_col = vec_p.tile([128, 1], FP32, tag="m_col")
    nc.vector.tensor_scalar(m_col[:], m_bc[:], 1.01, None, ALU.mult)
    negm_col = vec_p.tile([128, 1], FP32, tag="negm_col")
    nc.vector.tensor_scalar(negm_col[:], m_col[:], -1.0, None, ALU.mult)
    delta_col = vec_p.tile([128, 1], FP32, tag="delta_col")
    nc.vector.tensor_scalar(delta_col[:], m_col[:], 2.0 / K, None, ALU.mult)
    negdelta_col = vec_p.tile([128, 1], FP32, tag="negdelta_col")
    nc.vector.tensor_scalar(negdelta_col[:], m_col[:], -2.0 / K, None, ALU.mult)
    negt_col = vec_p.tile([128, 1], FP32, tag="negt_col")
    nc.vector.tensor_scalar(
        negt_col[:], iota_col[:], negdelta_col[:, 0:1], m_col[:, 0:1],
        ALU.mult, ALU.add,
    )
    iota_bc_ps = ps_tr.tile([128, 128], FP32, tag="trb")
    nc.tensor.matmul(
        iota_bc_ps[:], lhsT=ones_row_f[:], rhs=iota_row[:], start=True, stop=True
    )
    Tgrid_row = vec_p.tile([128, K], FP32, tag="Tgrid_row")
    nc.vector.tensor_scalar(
        Tgrid_row[:], iota_bc_ps[:, 0:K], delta_col[:, 0:1], negm_col[:, 0:1],
        ALU.mult, ALU.add,
    )


    # ---------------- phase C: S broadcasts (to SBUF), O1 + F1 -----------
    O1 = big_p.tile([128, NB, K], BF16, tag="O1")
    O2 = big_p.tile([128, NB, 128], BF16, tag="O2")
    S_row = big_p.tile([128, NB, 128], BF16, tag="S_row")
    for c in range(4):
        S_ps = ps_tr.tile([128, 512], FP32, tag="trb")
        nc.tensor.matmul(
            S_ps[:], lhsT=ones_row_b[:], rhs=s_flat[0:1, c * 512 : (c + 1) * 512],
            start=True, stop=True,
        )
        nc.scalar.copy(S_row[:, 4 * c : 4 * (c + 1), :], S_ps[:])
    F1_ps = ps_sm.tile([4, K], FP32, tag="sm")
    for a in range(NB):
        nc.vector.tensor_scalar(
            O1[:, a, :], Tgrid_row[:], s_mat[:, a : a + 1], None, ALU.is_lt
        )
        nc.tensor.matmul(
            F1_ps[:], lhsT=Pk[:, a, :], rhs=O1[:, a, :],
            start=(a == 0), stop=(a == NB - 1),
        )

    # h matmuls fill the PE behind the F1 sprint; h copied out of PSUM so
    # the output stage reads SBUF from both engines without serializing
    h_ps = ps_big.tile([128, NB, 128], FP32, tag="bigps")
    for t in range(NB):
        sl = slice(t * 128, (t + 1) * 128)
        nc.tensor.matmul(
            h_ps[:, t, :], lhsT=xT_hi[:, sl], rhs=W_hi[:], start=True, stop=True
        )
    h_sb = big_p.tile([128, NB, 128], FP32, tag="h_sb")
    for i2 in range(NB // 2):
        nc.scalar.copy(h_sb[:, 2 * i2 : 2 * i2 + 2, :], h_ps[:, 2 * i2 : 2 * i2 + 2, :])

    # Qtot / Ptot sums and broadcasts (needed at D assembly)
    qs = vec_p.tile([128, 1], FP32, tag="qs")
    nc.vector.reduce_sum(qs[:], q_v[:], axis=mybir.AxisListType.X)
    Qt_ps = ps_sm.tile([1, 1], FP32, tag="sm")
    nc.tensor.matmul(Qt_ps[:], lhsT=qs[:], rhs=ones_f[:, 0:1], start=True, stop=True)
    Qt_sb = vec_p.tile([1, 1], FP32, tag="Qt_sb")
    nc.vector.tensor_copy(Qt_sb[:], Qt_ps[:])
    nQt_ps = ps_sm.tile([128, 1], FP32, tag="sm")
    nc.tensor.matmul(nQt_ps[:], lhsT=neg_ones_row_f[:], rhs=Qt_sb[:], start=True, stop=True)
    negQt_col = vec_p.tile([128, 1], FP32, tag="negQt_col")
    nc.vector.tensor_copy(negQt_col[:], nQt_ps[:])

    # ---------------- D1 prep: diffs + transpose-combine -----------------
    def d_prep(F_ps, nm):
        F_sb = vec_p.tile([4, K], FP32, tag=nm + "F_sb")
        nc.vector.tensor_copy(F_sb[:], F_ps[:])
        D1_row = vec_p.tile([4, K], FP32, tag=nm + "D1_row")
        nc.vector.tensor_copy(D1_row[:, 0:1], F_sb[:, 0:1])
        nc.vector.tensor_tensor(
            D1_row[:, 1:K], F_sb[:, 1:K], F_sb[:, 0 : K - 1], ALU.subtract
        )
        Dcol_ps = ps_sm.tile([K, 2], FP32, tag="sm")
        nc.tensor.matmul(Dcol_ps[:], lhsT=D1_row[:], rhs=C42[:], start=True, stop=True)
        Dk = vec_p.tile([128, 4], BF16, tag=nm + "Dk")
        nc.gpsimd.memset(Dk[:], 0.0)
        nc.vector.tensor_copy(Dk[0:K, 0:1], Dcol_ps[:, 0:1])
        h32 = vec_p.tile([K, 2], FP32, tag=nm + "h32")
        nc.vector.tensor_copy(h32[:, 0:1], Dk[0:K, 0:1])
        nc.vector.tensor_tensor(Dk[0:K, 1:2], Dcol_ps[:, 0:1], h32[:, 0:1], ALU.subtract)
        nc.vector.tensor_copy(Dk[0:K, 2:3], Dcol_ps[:, 1:2])
        nc.vector.tensor_copy(h32[:, 1:2], Dk[0:K, 2:3])
        nc.vector.tensor_tensor(Dk[0:K, 3:4], Dcol_ps[:, 1:2], h32[:, 1:2], ALU.subtract)
        return Dk

    Dk1 = d_prep(F1_ps, "g1")

    # ------------- O2 comparisons interleaved with gather 1 --------------
    MPq_ps = ps_sm.tile([128, NB, 4], FP32, tag="sm")
    for c in range(4):
        nc.vector.tensor_scalar(
            O2[:, c * 4 : (c + 1) * 4, :],
            S_row[:, 4 * c : 4 * (c + 1), :],
            negt_col[:, 0:1], None, ALU.is_le,
        )
        for t in range(4 * c, 4 * c + 4):
            nc.tensor.matmul(
                MPq_ps[:, t, :], lhsT=O2[:, t, :], rhs=Dk1[:], start=True, stop=True
            )
    Dp = vec_p.tile([128, NB, 4], FP32, tag="Dp")
    # ------------- D assembly in halves; F2 starts after half 1 ----------
    MP = vec_p.tile([128, NB], FP32, tag="MP")
    MQ = vec_p.tile([128, NB], FP32, tag="MQ")
    t2 = vec_p.tile([128, NB], FP32, tag="t2")
    t3 = vec_p.tile([128, NB], FP32, tag="t3")
    D_v = vec_p.tile([128, NB], FP32, tag="D_v")
    invD = vec_p.tile([128, NB], FP32, tag="invD")
    r_v = vec_p.tile([128, NB], FP32, tag="r_v")
    u_v = vec_p.tile([128, NB], FP32, tag="u_v")
    Rk = vec_p.tile([128, NB, 4], BF16, tag="Rk")
    r_hi32 = vec_p.tile([128, NB], FP32, tag="r_hi32")
    u_hi32 = vec_p.tile([128, NB], FP32, tag="u_hi32")
    F2_ps = ps_sm.tile([4, K], FP32, tag="sm")
    H = NB // 2
    for half in range(2):
        hs = slice(half * H, (half + 1) * H)
        nc.vector.tensor_copy(Dp[:, hs, :], MPq_ps[:, hs, :])
        nc.vector.tensor_tensor(MP[:, hs], Dp[:, hs, 0], Dp[:, hs, 1], ALU.add)
        nc.vector.tensor_tensor(MQ[:, hs], Dp[:, hs, 2], Dp[:, hs, 3], ALU.add)
        nc.vector.scalar_tensor_tensor(
            t2[:, hs], MQ[:, hs], negQt_col[:, 0:1], q_v[:, hs], ALU.add, ALU.mult
        )
        nc.vector.tensor_tensor(t3[:, hs], p_v[:, hs], MP[:, hs], ALU.mult)
        nc.vector.tensor_tensor(D_v[:, hs], t3[:, hs], t2[:, hs], ALU.subtract)
        nc.vector.reciprocal(invD[:, hs], D_v[:, hs])
        nc.vector.tensor_tensor(r_v[:, hs], p_v[:, hs], invD[:, hs], ALU.mult)
        nc.vector.tensor_tensor(u_v[:, hs], q_v[:, hs], invD[:, hs], ALU.mult)
        nc.vector.tensor_copy(Rk[:, hs, 0], r_v[:, hs])
        nc.vector.tensor_copy(r_hi32[:, hs], Rk[:, hs, 0])
        nc.vector.tensor_tensor(Rk[:, hs, 1], r_v[:, hs], r_hi32[:, hs], ALU.subtract)
        nc.vector.tensor_copy(Rk[:, hs, 2], u_v[:, hs])
        nc.vector.tensor_copy(u_hi32[:, hs], Rk[:, hs, 2])
        nc.vector.tensor_tensor(Rk[:, hs, 3], u_v[:, hs], u_hi32[:, hs], ALU.subtract)
        for a in range(half * H, (half + 1) * H):
            nc.tensor.matmul(
                F2_ps[:], lhsT=Rk[:, a, :], rhs=O1[:, a, :],
                start=(a == 0), stop=(a == NB - 1),
            )
    us = vec_p.tile([128, 1], FP32, tag="us")
    nc.vector.reduce_sum(us[:], u_v[:], axis=mybir.AxisListType.X)

    # Utot scalar -> -Utot broadcast
    Ut_ps = ps_sm.tile([1, 1], FP32, tag="sm")
    nc.tensor.matmul(Ut_ps[:], lhsT=us[:], rhs=ones_f[:, 0:1], start=True, stop=True)
    Ut_sb = vec_p.tile([1, 1], FP32, tag="Ut_sb")
    nc.vector.tensor_copy(Ut_sb[:], Ut_ps[:])
    nUt_ps = ps_sm.tile([128, 1], FP32, tag="sm")
    nc.tensor.matmul(nUt_ps[:], lhsT=neg_ones_row_f[:], rhs=Ut_sb[:], start=True, stop=True)
    negUt_col = vec_p.tile([128, 1], FP32, tag="negUt_col")
    nc.vector.tensor_copy(negUt_col[:], nUt_ps[:])

    Dk2 = d_prep(F2_ps, "g2")

    # ---------------- gather 2 + output ----------------------------------
    out_view = out_d.rearrange("(t p) f -> p t f", p=128)
    Gp_ps = ps_sm.tile([128, NB, 4], FP32, tag="sm")
    Gp = vec_p.tile([128, NB, 4], FP32, tag="Gp")
    for t in range(NB):
        nc.tensor.matmul(
            Gp_ps[:, t, :], lhsT=O2[:, t, :], rhs=Dk2[:], start=True, stop=True
        )
    nc.scalar.copy(Gp[:], Gp_ps[:])
    MR = vec_p.tile([128, NB], FP32, tag="MR")
    MU = vec_p.tile([128, NB], FP32, tag="MU")
    z2 = vec_p.tile([128, NB], FP32, tag="z2")
    z3 = vec_p.tile([128, NB], FP32, tag="z3")
    col = vec_p.tile([128, NB], FP32, tag="col")
    col08 = vec_p.tile([128, NB], FP32, tag="col08")
    col02 = vec_p.tile([128, NB], FP32, tag="col02")
    nc.vector.tensor_tensor(MR[:], Gp[:, :, 0], Gp[:, :, 1], ALU.add)
    nc.vector.tensor_tensor(MU[:], Gp[:, :, 2], Gp[:, :, 3], ALU.add)
    nc.vector.scalar_tensor_tensor(
        z2[:], MU[:], negUt_col[:, 0:1], q_v[:], ALU.add, ALU.mult
    )
    nc.vector.tensor_tensor(z3[:], p_v[:], MR[:], ALU.mult)
    nc.vector.tensor_tensor(col[:], z3[:], z2[:], ALU.subtract)
    nc.vector.tensor_scalar(col08[:], col[:], 0.8, None, ALU.mult)
    nc.vector.tensor_scalar(col02[:], col[:], 0.2, None, ALU.mult)
    # out = col*lrelu(h) = relu(0.8*col*h) + 0.2*col*h   (col > 0)
    # pairs interleave ACT relu and DVE combine for cross-engine overlap
    r1_all = big_p.tile([128, NB, 128], FP32, tag="r1_all")
    for g in range(NB // 2):
        for t in (2 * g, 2 * g + 1):
            nc.scalar.activation(
                r1_all[:, t, :], h_sb[:, t, :], AFT.Relu,
                scale=col08[:, t : t + 1],
            )
        for t in (2 * g, 2 * g + 1):
            o_sb = outsb_p.tile([128, 128], FP32, tag="o_sb")
            nc.vector.scalar_tensor_tensor(
                o_sb[:], h_sb[:, t, :], col02[:, t : t + 1], r1_all[:, t, :],
                ALU.mult, ALU.add,
            )
            (nc.sync if t % 2 == 0 else nc.gpsimd).dma_start(
                out_view[:, t, :], o_sb[:]
            )


def build_nc(num_devices: int = 8) -> "bass.Bass":
    nc = bacc.Bacc(
        "TRN2", target_bir_lowering=False, debug=False, num_devices=num_devices
    )
    x_d = nc.dram_tensor("x", [N, F], FP32, kind="ExternalInput")
    W_d = nc.dram_tensor("W", [F, F], FP32, kind="ExternalInput")
    wm_d = nc.dram_tensor("w_mlp", [F], FP32, kind="ExternalInput")
    bm_d = nc.dram_tensor("b_mlp", [1], FP32, kind="ExternalInput")
    iota_d = nc.dram_tensor("iota", [128], FP32, kind="ExternalInput")
    c42_d = nc.dram_tensor("c42", [4, 2], FP32, kind="ExternalInput")
    out_d = nc.dram_tensor("out", [N, F], FP32, kind="ExternalOutput")
    with tile.TileContext(nc) as tc:
        with ExitStack() as ctx:
            gat_kernel(
                ctx, tc, out_d.ap(), x_d.ap(), W_d.ap(), wm_d.ap(), bm_d.ap(),
                iota_d.ap(), c42_d.ap(),
            )
    nc.compile()
    return nc


_NC_CACHE: dict = {}


def run(x, W, w_mlp, b_mlp, trace=False, **spmd_kwargs):
    x = np.asarray(x, dtype=np.float32)
    W = np.asarray(W, dtype=np.float32)
    w_mlp = np.asarray(w_mlp, dtype=np.float32)
    b_mlp = np.asarray(b_mlp, dtype=np.float32)

    if "nc" not in _NC_CACHE:
        _NC_CACHE["nc"] = build_nc(num_devices=B)
    nc = _NC_CACHE["nc"]

    iota = np.arange(128, dtype=np.float32)
    c42 = np.array([[1, 0], [1, 0], [0, 1], [0, 1]], dtype=np.float32)
    in_maps = [
        {
            "x": np.ascontiguousarray(x[b, 0]),
            "W": W,
            "w_mlp": w_mlp,
            "b_mlp": b_mlp,
            "iota": iota,
            "c42": c42,
        }
        for b in range(B)
    ]
    res = run_bass_kernel_spmd(
        nc, in_maps, core_ids=list(range(B)), trace=trace, **spmd_kwargs
    )
    out = np.stack([res.results[b]["out"] for b in range(B)])[:, None]
    return out.astype(np.float32), res


def kernel(x, W, w_mlp, b_mlp):
    out, _ = run(x, W, w_mlp, b_mlp)
    return out


# revision 6
# speedup vs baseline: 1.0938x; 1.0306x over previous
"""GAT layer kernel for Trainium2 (Bass/Tile), data-parallel over batch on 8 cores.

v9: histogram-gather reformulation — never materializes the [N,N] mask.

Per-core math (batch item b, N=2048, F=128, K=128 grid):
    s = x @ (W @ w_mlp) + b;  p = exp(s);  q = exp(0.2 s)
    grid t_k = -m + k*(2m/K), m = 1.01*max|s|
    O1[j,k] = [s_j > t_k]      (N x K comparisons)
    O2[k,i] = [s_i <= -t_k]    (K x N comparisons)
    F_w[k]  = sum_j O1[j,k] w_j            (tiny matmul, w in {p,q,r,u})
    (Mw)_i  ~= sum_k O2[k,i] (F_w[k]-F_w[k-1])   (gather matmul)
    D = p*(Mp) + q*(Qtot-(Mq));  r = p/D; u = q/D
    col = p*(Mr) + q*(Utot-(Mu));  out = lrelu(h) * col,  h = x @ W
Threshold quantization flips mask entries only in a |s_i+s_j| < delta band
where exp(lrelu(.)) is branch-insensitive, so the error is O(delta^2).

Phase layout minimizes cross-engine round trips: all PE transposes first
(casts chase on DVE/ACT), then s-matmuls (s ready early), h-matmuls fill
the PE while DVE builds the grid + O1/O2 comparisons.
"""

import sys

if "/opt/trn_rl_repo" not in sys.path:
    sys.path.insert(0, "/opt/trn_rl_repo")

from contextlib import ExitStack

import numpy as np

import concourse.bass as bass
import concourse.bass_isa as bass_isa
import concourse.mybir as mybir
import concourse.tile as tile
from concourse import bacc
from concourse import masks
from concourse.bass_utils import run_bass_kernel_spmd

B, N, F = 8, 2048, 128
NB = N // 128  # 16 token blocks
K = 32         # threshold grid size
NEG_SLOPE = 0.2
FP32 = mybir.dt.float32
BF16 = mybir.dt.bfloat16
ALU = mybir.AluOpType
AFT = mybir.ActivationFunctionType


def gat_kernel(ctx, tc, out_d, x_d, W_d, wm_d, bm_d, iota_d, c42_d):
    nc = tc.nc

    const_p = ctx.enter_context(tc.tile_pool(name="const", bufs=1))
    big_p = ctx.enter_context(tc.tile_pool(name="big", bufs=1))
    vec_p = ctx.enter_context(tc.tile_pool(name="vec", bufs=1))
    outsb_p = ctx.enter_context(tc.tile_pool(name="outsb", bufs=6))
    # PSUM: 8 banks. big=4 (h_ps, held to the end), tr=3 rotating, sm=1.
    ps_big = ctx.enter_context(tc.tile_pool(name="ps_big", bufs=1, space="PSUM"))
    ps_tr = ctx.enter_context(tc.tile_pool(name="ps_tr", bufs=3, space="PSUM"))
    ps_sm = ctx.enter_context(tc.tile_pool(name="ps_sm", bufs=1, space="PSUM"))

    # ---------------- input DMAs + gpsimd consts first ------------------
    W_sb = const_p.tile([128, 128], FP32, tag="W_sb")
    nc.sync.dma_start(W_sb[:], W_d[:, :])
    ident_f = const_p.tile([128, 128], FP32, tag="ident_f")
    ident_b = const_p.tile([128, 128], BF16, tag="ident_b")
    masks.make_identity(nc, ident_f[:])
    masks.make_identity(nc, ident_b[:])
    ones_f = const_p.tile([128, 1], FP32, tag="ones_f")
    nc.gpsimd.memset(ones_f[:], 1.0)
    ones_row_f = const_p.tile([1, 128], FP32, tag="ones_row_f")
    nc.gpsimd.memset(ones_row_f[:], 1.0)
    ones_row_b = const_p.tile([1, 128], BF16, tag="ones_row_b")
    nc.gpsimd.memset(ones_row_b[:], 1.0)
    neg_ones_row_f = const_p.tile([1, 128], FP32, tag="neg_ones_row_f")
    nc.gpsimd.memset(neg_ones_row_f[:], -1.0)

    x_view = x_d.rearrange("(t p) f -> p t f", p=128)
    x_sb = big_p.tile([128, NB, 128], FP32, tag="x_sb")
    x_q_of = {2: nc.scalar, 8: nc.scalar, 14: nc.scalar}
    x_alt = [nc.sync, nc.gpsimd]
    n_alt = 0
    for t in range(NB):
        q = x_q_of.get(t)
        if q is None:
            q = x_alt[n_alt % 2]
            n_alt += 1
        q.dma_start(x_sb[:, t, :], x_view[:, t, :])
    wm_sb = const_p.tile([128, 1], FP32, tag="wm_sb")
    nc.scalar.dma_start(wm_sb[:], wm_d.rearrange("(p o) -> p o", o=1))
    b_sb = const_p.tile([1, 1], FP32, tag="b_sb")
    nc.scalar.dma_start(b_sb[:], bm_d.rearrange("(p o) -> p o", o=1))
    iota_col = const_p.tile([128, 1], FP32, tag="iota_col")
    nc.scalar.dma_start(iota_col[:], iota_d.rearrange("(p o) -> p o", o=1))
    iota_row = const_p.tile([1, 128], FP32, tag="iota_row")
    nc.scalar.dma_start(iota_row[:], iota_d.rearrange("(o k) -> o k", o=1))
    C42 = const_p.tile([4, 2], FP32, tag="C42")
    nc.scalar.dma_start(C42[:], c42_d[:, :])

    # ACT tables (Exp for p/q, Relu for the output stage)
    warm = const_p.tile([128, 1], FP32, tag="warm")
    nc.scalar.activation(warm[:], ones_f[:], AFT.Exp)
    nc.scalar.activation(warm[:], ones_f[:], AFT.Relu)

    # b broadcast to [128,1] via K=1 PE matmul
    b_ps = ps_sm.tile([128, 1], FP32, tag="sm")
    nc.tensor.matmul(b_ps[:], lhsT=ones_row_f[:], rhs=b_sb[:], start=True, stop=True)
    b_bc = const_p.tile([128, 1], FP32, tag="b_bc")
    nc.vector.tensor_copy(b_bc[:], b_ps[:])

    # ---------------- W -> bf16; v = W @ w_mlp; vk pair ------------------
    W_hi = const_p.tile([128, 128], BF16, tag="W_hi")
    nc.scalar.copy(W_hi[:], W_sb[:])
    WT_ps = ps_sm.tile([128, 128], FP32, tag="sm")
    nc.tensor.transpose(WT_ps[:], W_sb[:], ident_f[:])
    WT_sb = vec_p.tile([128, 128], FP32, tag="WT_sb")
    nc.vector.tensor_copy(WT_sb[:], WT_ps[:])
    v_ps = ps_sm.tile([128, 1], FP32, tag="sm")
    nc.tensor.matmul(v_ps[:], lhsT=WT_sb[:], rhs=wm_sb[:], start=True, stop=True)
    v_sb = vec_p.tile([128, 1], FP32, tag="v_sb")
    nc.vector.tensor_copy(v_sb[:], v_ps[:])
    vk = vec_p.tile([128, 1], BF16, tag="vk")
    nc.vector.tensor_copy(vk[:], v_sb[:])
    b02 = vec_p.tile([128, 1], FP32, tag="b02")
    nc.vector.tensor_scalar(b02[:], b_bc[:], NEG_SLOPE, None, ALU.mult)

    # -------- phase A: transposes with s-matmuls interleaved (lag 2) -----
    xT_hi = big_p.tile([128, N], BF16, tag="xT_hi")  # [f, tok]
    s4_ps = ps_sm.tile([128, NB, 1], FP32, tag="sm")

    def s_mm(t):
        sl = slice(t * 128, (t + 1) * 128)
        nc.tensor.matmul(
            s4_ps[:, t, :], lhsT=xT_hi[:, sl], rhs=vk[:], start=True, stop=True
        )

    for t in range(NB):
        sl = slice(t * 128, (t + 1) * 128)
        tr32 = ps_tr.tile([128, 128], FP32, tag="trb")
        nc.tensor.transpose(tr32[:], x_sb[:, t, :], ident_f[:])
        if t % 2 == 0:
            nc.vector.tensor_copy(xT_hi[:, sl], tr32[:])
        else:
            nc.scalar.copy(xT_hi[:, sl], tr32[:])
        if t >= 2:
            s_mm(t - 2)
    s_mm(NB - 2)
    s_mm(NB - 1)
    s_mat = vec_p.tile([128, NB], FP32, tag="s_mat")
    nc.vector.tensor_scalar(s_mat[:], s4_ps[:, :, 0], b_bc[:, 0:1], None, ALU.add)
    s_hi = vec_p.tile([128, NB], BF16, tag="s_hi")
    nc.vector.tensor_copy(s_hi[:], s_mat[:])
    sT_ps = ps_tr.tile([16, 128], BF16, tag="trb")
    nc.tensor.transpose(sT_ps[:], s_hi[:], ident_b[:])
    sT_sb = vec_p.tile([16, 128], BF16, tag="sT_sb")
    nc.vector.tensor_copy(sT_sb[:], sT_ps[:])
    s_flat = vec_p.tile([1, N], BF16, tag="s_flat")
    nc.sync.dma_start(s_flat[0:1, :], sT_sb[:, :])

    # grid: m = 1.01*max|s|, thresholds along free (Tgrid) and partitions
    amax = vec_p.tile([128, 1], FP32, tag="amax")
    nc.vector.tensor_reduce(
        amax[:], s_mat[:], axis=mybir.AxisListType.X, op=ALU.max,
        apply_absolute_value=True,
    )
    amaxT_ps = ps_tr.tile([1, 128], FP32, tag="trb")
    nc.tensor.transpose(amaxT_ps[:], amax[:], ident_f[:])
    # exps + Pk splits fill the DVE while the partition-max round-trips PE
    p_v = vec_p.tile([128, NB], FP32, tag="p_v")
    nc.scalar.activation(p_v[:], s4_ps[:, :, 0], AFT.Exp, bias=b_bc[:, 0:1])
    q_v = vec_p.tile([128, NB], FP32, tag="q_v")
    nc.scalar.activation(
        q_v[:], s4_ps[:, :, 0], AFT.Exp, scale=NEG_SLOPE, bias=b02[:, 0:1]
    )
    Pk = vec_p.tile([128, NB, 4], BF16, tag="Pk")
    nc.vector.tensor_copy(Pk[:, :, 0], p_v[:])
    p_hi32 = vec_p.tile([128, NB], FP32, tag="p_hi32")
    nc.vector.tensor_copy(p_hi32[:], Pk[:, :, 0])
    nc.vector.tensor_tensor(Pk[:, :, 1], p_v[:], p_hi32[:], ALU.subtract)
    nc.vector.tensor_copy(Pk[:, :, 2], q_v[:])
    q_hi32 = vec_p.tile([128, NB], FP32, tag="q_hi32")
    nc.vector.tensor_copy(q_hi32[:], Pk[:, :, 2])
    nc.vector.tensor_tensor(Pk[:, :, 3], q_v[:], q_hi32[:], ALU.subtract)
    amaxT = vec_p.tile([1, 128], FP32, tag="amaxT")
    nc.vector.tensor_copy(amaxT[:], amaxT_ps[:])
    m_sc = vec_p.tile([1, 1], FP32, tag="m_sc")
    nc.vector.tensor_reduce(m_sc[:], amaxT[:], axis=mybir.AxisListType.X, op=ALU.max)
    m_ps = ps_sm.tile([128, 1], FP32, tag="sm")
    nc.tensor.matmul(m_ps[:], lhsT=ones_row_f[:], rhs=m_sc[:], start=True, stop=True)
    m_bc = vec_p.tile([128, 1], FP32, tag="m_bc")
    nc.vector.tensor_copy(m_bc[:], m_ps[:])
    m_col = vec_p.tile([128, 1], FP32, tag="m_col")
    nc.vector.tensor_scalar(m_col[:], m_bc[:], 1.01, None, ALU.mult)
    negm_col = vec_p.tile([128, 1], FP32, tag="negm_col")
    nc.vector.tensor_scalar(negm_col[:], m_col[:], -1.0, None, ALU.mult)
    delta_col = vec_p.tile([128, 1], FP32, tag="delta_col")
    nc.vector.tensor_scalar(delta_col[:], m_col[:], 2.0 / K, None, ALU.mult)
    negdelta_col = vec_p.tile([128, 1], FP32, tag="negdelta_col")
    nc.vector.tensor_scalar(negdelta_col[:], m_col[:], -2.0 / K, None, ALU.mult)
    negt_col = vec_p.tile([128, 1], FP32, tag="negt_col")
    nc.vector.tensor_scalar(
        negt_col[:], iota_col[:], negdelta_col[:, 0:1], m_col[:, 0:1],
        ALU.mult, ALU.add,
    )
    iota_bc_ps = ps_tr.tile([128, 128], FP32, tag="trb")
    nc.tensor.matmul(
        iota_bc_ps[:], lhsT=ones_row_f[:], rhs=iota_row[:], start=True, stop=True
    )
    Tgrid_row = vec_p.tile([128, K], FP32, tag="Tgrid_row")
    nc.vector.tensor_scalar(
        Tgrid_row[:], iota_bc_ps[:, 0:K], delta_col[:, 0:1], negm_col[:, 0:1],
        ALU.mult, ALU.add,
    )


    # ---------------- phase C: S broadcasts (to SBUF), O1 + F1 -----------
    O1 = big_p.tile([128, NB, K], BF16, tag="O1")
    O2 = big_p.tile([128, NB, 128], BF16, tag="O2")
    S_row = big_p.tile([128, NB, 128], BF16, tag="S_row")
    for c in range(4):
        S_ps = ps_tr.tile([128, 512], FP32, tag="trb")
        nc.tensor.matmul(
            S_ps[:], lhsT=ones_row_b[:], rhs=s_flat[0:1, c * 512 : (c + 1) * 512],
            start=True, stop=True,
        )
        nc.scalar.copy(S_row[:, 4 * c : 4 * (c + 1), :], S_ps[:])
    F1_ps = ps_sm.tile([4, K], FP32, tag="sm")
    for a in range(NB):
        nc.vector.tensor_scalar(
            O1[:, a, :], Tgrid_row[:], s_mat[:, a : a + 1], None, ALU.is_lt
        )
        nc.tensor.matmul(
            F1_ps[:], lhsT=Pk[:, a, :], rhs=O1[:, a, :],
            start=(a == 0), stop=(a == NB - 1),
        )

    # h matmuls fill the PE behind the F1 sprint; h copied out of PSUM so
    # the output stage reads SBUF from both engines without serializing
    h_ps = ps_big.tile([128, NB, 128], FP32, tag="bigps")
    for t in range(NB):
        sl = slice(t * 128, (t + 1) * 128)
        nc.tensor.matmul(
            h_ps[:, t, :], lhsT=xT_hi[:, sl], rhs=W_hi[:], start=True, stop=True
        )
    h_sb = big_p.tile([128, NB, 128], FP32, tag="h_sb")
    for i2 in range(NB // 2):
        nc.scalar.copy(h_sb[:, 2 * i2 : 2 * i2 + 2, :], h_ps[:, 2 * i2 : 2 * i2 + 2, :])

    # Qtot / Ptot sums and broadcasts (needed at D assembly)
    qs = vec_p.tile([128, 1], FP32, tag="qs")
    nc.vector.reduce_sum(qs[:], q_v[:], axis=mybir.AxisListType.X)
    Qt_ps = ps_sm.tile([1, 1], FP32, tag="sm")
    nc.tensor.matmul(Qt_ps[:], lhsT=qs[:], rhs=ones_f[:, 0:1], start=True, stop=True)
    Qt_sb = vec_p.tile([1, 1], FP32, tag="Qt_sb")
    nc.vector.tensor_copy(Qt_sb[:], Qt_ps[:])
    nQt_ps = ps_sm.tile([128, 1], FP32, tag="sm")
    nc.tensor.matmul(nQt_ps[:], lhsT=neg_ones_row_f[:], rhs=Qt_sb[:], start=True, stop=True)
    negQt_col = vec_p.tile([128, 1], FP32, tag="negQt_col")
    nc.vector.tensor_copy(negQt_col[:], nQt_ps[:])

    # ---------------- D1 prep: diffs + transpose-combine -----------------
    def d_prep(F_ps, nm):
        F_sb = vec_p.tile([4, K], FP32, tag=nm + "F_sb")
        nc.vector.tensor_copy(F_sb[:], F_ps[:])
        D1_row = vec_p.tile([4, K], FP32, tag=nm + "D1_row")
        nc.vector.tensor_copy(D1_row[:, 0:1], F_sb[:, 0:1])
        nc.vector.tensor_tensor(
            D1_row[:, 1:K], F_sb[:, 1:K], F_sb[:, 0 : K - 1], ALU.subtract
        )
        Dcol_ps = ps_sm.tile([K, 2], FP32, tag="sm")
        nc.tensor.matmul(Dcol_ps[:], lhsT=D1_row[:], rhs=C42[:], start=True, stop=True)
        Dk = vec_p.tile([128, 4], BF16, tag=nm + "Dk")
        nc.gpsimd.memset(Dk[:], 0.0)
        nc.vector.tensor_copy(Dk[0:K, 0:1], Dcol_ps[:, 0:1])
        h32 = vec_p.tile([K, 2], FP32, tag=nm + "h32")
        nc.vector.tensor_copy(h32[:, 0:1], Dk[0:K, 0:1])
        nc.vector.tensor_tensor(Dk[0:K, 1:2], Dcol_ps[:, 0:1], h32[:, 0:1], ALU.subtract)
        nc.vector.tensor_copy(Dk[0:K, 2:3], Dcol_ps[:, 1:2])
        nc.vector.tensor_copy(h32[:, 1:2], Dk[0:K, 2:3])
        nc.vector.tensor_tensor(Dk[0:K, 3:4], Dcol_ps[:, 1:2], h32[:, 1:2], ALU.subtract)
        return Dk

    Dk1 = d_prep(F1_ps, "g1")

    # ------------- O2 comparisons interleaved with gather 1 --------------
    MPq_ps = ps_sm.tile([128, NB, 4], FP32, tag="sm")
    for c in range(4):
        nc.vector.tensor_scalar(
            O2[:, c * 4 : (c + 1) * 4, :],
            S_row[:, 4 * c : 4 * (c + 1), :],
            negt_col[:, 0:1], None, ALU.is_le,
        )
        for t in range(4 * c, 4 * c + 4):
            nc.tensor.matmul(
                MPq_ps[:, t, :], lhsT=O2[:, t, :], rhs=Dk1[:], start=True, stop=True
            )
    Dp = vec_p.tile([128, NB, 4], FP32, tag="Dp")
    # ------------- D assembly in halves; F2 starts after half 1 ----------
    MP = vec_p.tile([128, NB], FP32, tag="MP")
    MQ = vec_p.tile([128, NB], FP32, tag="MQ")
    t2 = vec_p.tile([128, NB], FP32, tag="t2")
    t3 = vec_p.tile([128, NB], FP32, tag="t3")
    D_v = vec_p.tile([128, NB], FP32, tag="D_v")
    invD = vec_p.tile([128, NB], FP32, tag="invD")
    r_v = vec_p.tile([128, NB], FP32, tag="r_v")
    u_v = vec_p.tile([128, NB], FP32, tag="u_v")
    Rk = vec_p.tile([128, NB, 4], BF16, tag="Rk")
    r_hi32 = vec_p.tile([128, NB], FP32, tag="r_hi32")
    u_hi32 = vec_p.tile([128, NB], FP32, tag="u_hi32")
    F2_ps = ps_sm.tile([4, K], FP32, tag="sm")
    H = NB // 2
    for half in range(2):
        hs = slice(half * H, (half + 1) * H)
        nc.vector.tensor_copy(Dp[:, hs, :], MPq_ps[:, hs, :])
        nc.vector.tensor_tensor(MP[:, hs], Dp[:, hs, 0], Dp[:, hs, 1], ALU.add)
        nc.vector.tensor_tensor(MQ[:, hs], Dp[:, hs, 2], Dp[:, hs, 3], ALU.add)
        nc.vector.scalar_tensor_tensor(
            t2[:, hs], MQ[:, hs], negQt_col[:, 0:1], q_v[:, hs], ALU.add, ALU.mult
        )
        nc.vector.tensor_tensor(t3[:, hs], p_v[:, hs], MP[:, hs], ALU.mult)
        nc.vector.tensor_tensor(D_v[:, hs], t3[:, hs], t2[:, hs], ALU.subtract)
        nc.vector.reciprocal(invD[:, hs], D_v[:, hs])
        nc.vector.tensor_tensor(r_v[:, hs], p_v[:, hs], invD[:, hs], ALU.mult)
        nc.vector.tensor_tensor(u_v[:, hs], q_v[:, hs], invD[:, hs], ALU.mult)
        nc.vector.tensor_copy(Rk[:, hs, 0], r_v[:, hs])
        nc.vector.tensor_copy(r_hi32[:, hs], Rk[:, hs, 0])
        nc.vector.tensor_tensor(Rk[:, hs, 1], r_v[:, hs], r_hi32[:, hs], ALU.subtract)
        nc.vector.tensor_copy(Rk[:, hs, 2], u_v[:, hs])
        nc.vector.tensor_copy(u_hi32[:, hs], Rk[:, hs, 2])
        nc.vector.tensor_tensor(Rk[:, hs, 3], u_v[:, hs], u_hi32[:, hs], ALU.subtract)
        for a in range(half * H, (half + 1) * H):
            nc.tensor.matmul(
                F2_ps[:], lhsT=Rk[:, a, :], rhs=O1[:, a, :],
                start=(a == 0), stop=(a == NB - 1),
            )
    us = vec_p.tile([128, 1], FP32, tag="us")
    nc.vector.reduce_sum(us[:], u_v[:], axis=mybir.AxisListType.X)

    # Utot scalar -> -Utot broadcast
    Ut_ps = ps_sm.tile([1, 1], FP32, tag="sm")
    nc.tensor.matmul(Ut_ps[:], lhsT=us[:], rhs=ones_f[:, 0:1], start=True, stop=True)
    Ut_sb = vec_p.tile([1, 1], FP32, tag="Ut_sb")
    nc.vector.tensor_copy(Ut_sb[:], Ut_ps[:])
    nUt_ps = ps_sm.tile([128, 1], FP32, tag="sm")
    nc.tensor.matmul(nUt_ps[:], lhsT=neg_ones_row_f[:], rhs=Ut_sb[:], start=True, stop=True)
    negUt_col = vec_p.tile([128, 1], FP32, tag="negUt_col")
    nc.vector.tensor_copy(negUt_col[:], nUt_ps[:])

    Dk2 = d_prep(F2_ps, "g2")

    # ---------------- gather 2 + output ----------------------------------
    out_view = out_d.rearrange("(t p) f -> p t f", p=128)
    Gp_ps = ps_sm.tile([128, NB, 4], FP32, tag="sm")
    Gp = vec_p.tile([128, NB, 4], FP32, tag="Gp")
    for t in range(NB):
        nc.tensor.matmul(
            Gp_ps[:, t, :], lhsT=O2[:, t, :], rhs=Dk2[:], start=True, stop=True
        )
    nc.scalar.copy(Gp[:], Gp_ps[:])
    MR = vec_p.tile([128, NB], FP32, tag="MR")
    MU = vec_p.tile([128, NB], FP32, tag="MU")
    z2 = vec_p.tile([128, NB], FP32, tag="z2")
    z3 = vec_p.tile([128, NB], FP32, tag="z3")
    col = vec_p.tile([128, NB], FP32, tag="col")
    col08 = vec_p.tile([128, NB], FP32, tag="col08")
    col02 = vec_p.tile([128, NB], FP32, tag="col02")
    nc.vector.tensor_tensor(MR[:], Gp[:, :, 0], Gp[:, :, 1], ALU.add)
    nc.vector.tensor_tensor(MU[:], Gp[:, :, 2], Gp[:, :, 3], ALU.add)
    nc.vector.scalar_tensor_tensor(
        z2[:], MU[:], negUt_col[:, 0:1], q_v[:], ALU.add, ALU.mult
    )
    nc.vector.tensor_tensor(z3[:], p_v[:], MR[:], ALU.mult)
    nc.vector.tensor_tensor(col[:], z3[:], z2[:], ALU.subtract)
    nc.vector.tensor_scalar(col08[:], col[:], 0.8, None, ALU.mult)
    nc.vector.tensor_scalar(col02[:], col[:], 0.2, None, ALU.mult)
    # out = col*lrelu(h) = relu(0.8*col*h) + 0.2*col*h   (col > 0)
    # pairs interleave ACT relu and DVE combine for cross-engine overlap
    r1_all = big_p.tile([128, NB, 128], FP32, tag="r1_all")
    for g in range(NB // 2):
        for t in (2 * g, 2 * g + 1):
            nc.scalar.activation(
                r1_all[:, t, :], h_sb[:, t, :], AFT.Relu,
                scale=col08[:, t : t + 1],
            )
        for t in (2 * g, 2 * g + 1):
            o_sb = outsb_p.tile([128, 128], FP32, tag="o_sb")
            nc.vector.scalar_tensor_tensor(
                o_sb[:], h_sb[:, t, :], col02[:, t : t + 1], r1_all[:, t, :],
                ALU.mult, ALU.add,
            )
            (nc.sync if t % 2 == 0 else nc.gpsimd).dma_start(
                out_view[:, t, :], o_sb[:]
            )


def build_nc(num_devices: int = 8) -> "bass.Bass":
    nc = bacc.Bacc(
        "TRN2", target_bir_lowering=False, debug=False, num_devices=num_devices
    )
    x_d = nc.dram_tensor("x", [N, F], FP32, kind="ExternalInput")
    W_d = nc.dram_tensor("W", [F, F], FP32, kind="ExternalInput")
    wm_d = nc.dram_tensor("w_mlp", [F], FP32, kind="ExternalInput")
    bm_d = nc.dram_tensor("b_mlp", [1], FP32, kind="ExternalInput")
    iota_d = nc.dram_tensor("iota", [128], FP32, kind="ExternalInput")
    c42_d = nc.dram_tensor("c42", [4, 2], FP32, kind="ExternalInput")
    out_d = nc.dram_tensor("out", [N, F], FP32, kind="ExternalOutput")
    with tile.TileContext(nc) as tc:
        with ExitStack() as ctx:
            gat_kernel(
                ctx, tc, out_d.ap(), x_d.ap(), W_d.ap(), wm_d.ap(), bm_d.ap(),
                iota_d.ap(), c42_d.ap(),
            )
    nc.compile()
    return nc


_NC_CACHE: dict = {}


def run(x, W, w_mlp, b_mlp, trace=False, **spmd_kwargs):
    x = np.asarray(x, dtype=np.float32)
    W = np.asarray(W, dtype=np.float32)
    w_mlp = np.asarray(w_mlp, dtype=np.float32)
    b_mlp = np.asarray(b_mlp, dtype=np.float32)

    if "nc" not in _NC_CACHE:
        _NC_CACHE["nc"] = build_nc(num_devices=B)
    nc = _NC_CACHE["nc"]

    iota = np.arange(128, dtype=np.float32)
    c42 = np.array([[1, 0], [1, 0], [0, 1], [0, 1]], dtype=np.float32)
    in_maps = [
        {
            "x": np.ascontiguousarray(x[b, 0]),
            "W": W,
            "w_mlp": w_mlp,
            "b_mlp": b_mlp,
            "iota": iota,
            "c42": c42,
        }
        for b in range(B)
    ]
    res = run_bass_kernel_spmd(
        nc, in_maps, core_ids=list(range(B)), trace=trace, **spmd_kwargs
    )
    out = np.stack([res.results[b]["out"] for b in range(B)])[:, None]
    return out.astype(np.float32), res


def kernel(x, W, w_mlp, b_mlp):
    out, _ = run(x, W, w_mlp, b_mlp)
    return out
